# revision 13
# baseline (speedup 1.0000x reference)
"""Multi-head causal attention (B=4, S=2048, D=1024, H=16, HD=64) on 8 TRN2 cores.

Strategy:
  - Head-parallel: core i computes heads {2i, 2i+1} for all tokens.
    Host pre-transposes x -> xT [D, B*S], folds the 1/sqrt(HD) scale into Wq,
    converts matmul inputs to bf16, and adds bo at the end.
  - On device per core and per batch: qT/kT/vT projections (weights
    stationary, xT moving), scores computed transposed [k, q] with the two
    heads packed via PE row tiling (K=64 each), exp on ACT, PV matmul with
    stationary [v | 1] so the softmax denominator lands in output row 64,
    normalize via fast reciprocal + gpsimd partition_broadcast.
  - One AllToAll per batch (tokens of that batch, interleaved token-tile ->
    rank mapping) reshards head-outputs feature-major; the output projection
    for those tokens runs overlapped with the next batch's attention.
"""

import sys

sys.path.insert(0, "/opt/trn_rl_repo")

import numpy as np

import concourse.bass as bass
import concourse.mybir as mybir
import concourse.tile as tile
from concourse import bacc, bass_utils

FP = mybir.dt.float32
BF = mybir.dt.bfloat16
AOP = mybir.AluOpType
AFT = mybir.ActivationFunctionType

B, S, D, H = 4, 2048, 1024, 16
HD = 64
N_CORES = 8
NT = B * S  # 8192 tokens
TOK_PER_CORE = NT // N_CORES  # 1024
KD = D // 128  # 8 contraction tiles for the projections


def build_nc():
    nc = bacc.Bacc(None, target_bir_lowering=False, debug=False, num_devices=N_CORES)

    xt = nc.dram_tensor("xt", [D, NT], BF, kind="ExternalInput")
    wqk = nc.dram_tensor("wqk", [2, D, 128], BF, kind="ExternalInput")
    wv = nc.dram_tensor("wv", [D, 128], BF, kind="ExternalInput")
    bqk = nc.dram_tensor("bqk", [2, 128, 1], FP, kind="ExternalInput")
    bvb = nc.dram_tensor("bv", [128, 1], FP, kind="ExternalInput")
    wo = nc.dram_tensor("wo", [D, D], BF, kind="ExternalInput")
    maskd = nc.dram_tensor("mask", [128, 896], BF, kind="ExternalInput")
    identd = nc.dram_tensor("ident", [128, 128], BF, kind="ExternalInput")
    out = nc.dram_tensor("out", [TOK_PER_CORE, D], FP, kind="ExternalOutput")

    with tile.TileContext(nc) as tc:
        with (
            tc.tile_pool(name="const", bufs=1) as const,
            tc.tile_pool(name="xtp", bufs=12) as xtp,
            tc.tile_pool(name="qkv", bufs=2) as qkv,
            tc.tile_pool(name="vnp", bufs=18) as vnp,
            tc.tile_pool(name="esp", bufs=3) as esp,
            tc.tile_pool(name="small", bufs=4) as small,
            tc.tile_pool(name="onp", bufs=4) as onp,
            tc.tile_pool(name="actp", bufs=10) as actp,
            tc.tile_pool(name="oop", bufs=3) as oop,
            tc.tile_pool(name="ps_mm", bufs=2, space="PSUM") as ps_mm,
            tc.tile_pool(name="ps_s", bufs=2, space="PSUM") as ps_s,
            tc.tile_pool(name="ps_o", bufs=1, space="PSUM") as ps_o,
            tc.tile_pool(name="dram", bufs=1, space="DRAM") as dram,
        ):
            cc_ins = [
                dram.tile([N_CORES, 128, 256], BF, name=f"cc_in{b}") for b in range(B)
            ]
            cc_outs = [
                dram.tile([N_CORES, 128, 256], BF, name=f"cc_out{b}") for b in range(B)
            ]

            # ---- resident constants ----
            mask_sb = const.tile([128, 896], BF, name="mask_sb")
            nc.sync.dma_start(mask_sb[:], maskd[:])
            ident_sb = const.tile([128, 128], BF, name="ident_sb")
            nc.sync.dma_start(ident_sb[:], identd[:])
            wqk_sb = const.tile([128, 2 * KD, 128], BF, name="wqk_sb")
            nc.sync.dma_start(
                wqk_sb[:],
                wqk.rearrange("h (ko p) m -> p (h ko) m", p=128),
            )
            wv_sb = const.tile([128, KD, 128], BF, name="wv_sb")
            nc.sync.dma_start(wv_sb[:], wv.rearrange("(ko p) m -> p ko m", p=128))
            bqk_sb = const.tile([128, 2], FP, name="bqk_sb")
            nc.sync.dma_start(bqk_sb[:], bqk.rearrange("h p one -> p (h one)"))
            bv_sb = const.tile([128, 1], FP, name="bv_sb")
            nc.sync.dma_start(bv_sb[:], bvb[:])
            wo_sb = const.tile([128, KD, D], BF, name="wo_sb")
            nc.gpsimd.dma_start(wo_sb[:], wo.rearrange("(ko p) n -> p ko n", p=128))

            for b in range(B):
                # ---- QKV projection for this batch (both heads) ----
                qT = qkv.tile([128, S], BF, name="qT", tag="qT")  # h0 rows 0-63, h1 64-127
                kT = qkv.tile([128, S], BF, name="kT", tag="kT")
                vT = qkv.tile([128, S], BF, name="vT", tag="vT")
                for st in range(4):  # 512-token slabs
                    xts = []
                    for kd in range(KD):
                        xt_t = xtp.tile([128, 512], BF, name="xt_t", tag="xt")
                        nc.sync.dma_start(
                            xt_t[:],
                            xt[kd * 128 : (kd + 1) * 128, b * S + st * 512 : b * S + (st + 1) * 512],
                        )
                        xts.append(xt_t)
                    for h in range(2):
                        ps = ps_mm.tile([128, 512], FP, name="ps_qk", tag="mm")
                        for kd in range(KD):
                            nc.tensor.matmul(
                                ps[:],
                                lhsT=wqk_sb[:, h * KD + kd, :],
                                rhs=xts[kd][:],
                                start=(kd == 0),
                                stop=(kd == KD - 1),
                            )
                        nc.vector.tensor_scalar(
                            qT[h * 64 : h * 64 + 64, st * 512 : (st + 1) * 512],
                            ps[0:64, :],
                            bqk_sb[0:64, h : h + 1],
                            None,
                            AOP.add,
                        )
                        nc.vector.tensor_scalar(
                            kT[h * 64 : h * 64 + 64, st * 512 : (st + 1) * 512],
                            ps[64:128, :],
                            bqk_sb[64:128, h : h + 1],
                            None,
                            AOP.add,
                        )
                    ps = ps_mm.tile([128, 512], FP, name="ps_v", tag="mm")
                    for kd in range(KD):
                        nc.tensor.matmul(
                            ps[:],
                            lhsT=wv_sb[:, kd, :],
                            rhs=xts[kd][:],
                            start=(kd == 0),
                            stop=(kd == KD - 1),
                        )
                    nc.vector.tensor_scalar(
                        vT[:, st * 512 : (st + 1) * 512],
                        ps[:],
                        bv_sb[:, 0:1],
                        None,
                        AOP.add,
                    )

                # ---- vT -> v natural [token, hd] with ones columns ----
                vn_tiles = []
                for kc in range(S // 128):
                    pst = ps_mm.tile([128, 128], BF, name="ps_t", tag="mm")
                    nc.tensor.transpose(pst[:], vT[:, kc * 128 : (kc + 1) * 128], ident_sb[:])
                    vn = vnp.tile([128, 130], BF, name="vn", tag="vn")
                    nc.vector.tensor_copy(out=vn[:, 0:64], in_=pst[:, 0:64])
                    nc.vector.tensor_copy(out=vn[:, 65:129], in_=pst[:, 64:128])
                    nc.vector.tensor_copy(out=vn[:, 64:65], in_=mask_sb[:, 895:896])
                    nc.vector.tensor_copy(out=vn[:, 129:130], in_=mask_sb[:, 895:896])
                    vn_tiles.append(vn)

                # ---- causal attention, scores transposed [k, q] ----
                for qi in range(4):  # 512-wide query tiles
                    po = [
                        ps_o.tile([65, 512], FP, name=f"po{h}", tag=f"o{h}")
                        for h in range(2)
                    ]
                    nki = 4 * (qi + 1)
                    for ki in range(nki):
                        pss = ps_s.tile([128, 1024], FP, name="ps_sc", tag="sc")
                        for h in range(2):
                            nc.tensor.matmul(
                                pss[:, h * 512 : (h + 1) * 512],
                                lhsT=kT[h * 64 : h * 64 + 64, ki * 128 : (ki + 1) * 128],
                                rhs=qT[h * 64 : h * 64 + 64, qi * 512 : (qi + 1) * 512],
                                start=True,
                                stop=True,
                                tile_position=(h * 64, 0),
                            )
                        es = esp.tile([128, 1024], BF, name="es", tag="es")
                        nc.scalar.activation(es[:], pss[:], AFT.Exp)
                        if ki >= 4 * qi:  # diagonal tile: multiplicative causal mask
                            j = ki - 4 * qi
                            for h in range(2):
                                nc.vector.tensor_tensor(
                                    es[:, h * 512 : (h + 1) * 512],
                                    es[:, h * 512 : (h + 1) * 512],
                                    mask_sb[:, 384 - 128 * j : 896 - 128 * j],
                                    AOP.mult,
                                )
                        for h in range(2):
                            nc.tensor.matmul(
                                po[h][:],
                                lhsT=vn_tiles[ki][:, h * 65 : (h + 1) * 65],
                                rhs=es[:, h * 512 : (h + 1) * 512],
                                start=(ki == 0),
                                stop=(ki == nki - 1),
                            )
                    # normalize and scatter into this batch's A2A send buffer
                    for h in range(2):
                        oc = small.tile([65, 512], FP, name="oc", tag="oc")
                        nc.vector.tensor_copy(out=oc[:], in_=po[h][:])
                        den = small.tile([1, 512], FP, name="den", tag="den")
                        nc.vector.tensor_copy(out=den[:], in_=oc[64:65, :])
                        bc = small.tile([64, 512], FP, name="bc", tag="bc")
                        nc.gpsimd.partition_broadcast(bc[:], den[0:1, :], channels=64)
                        rc = small.tile([64, 512], FP, name="rc", tag="rc")
                        nc.vector.reciprocal_approx_fast(out=rc[:], in_=bc[:])
                        on = onp.tile([64, 512], BF, name="on", tag="on")
                        nc.vector.tensor_tensor(on[:], oc[0:64, :], rc[:], AOP.mult)
                        for i in range(4):
                            t = 4 * qi + i  # token tile within batch (0..15)
                            rank = t % 8
                            pos = 128 * (t // 8)
                            nc.sync.dma_start(
                                cc_ins[b][rank, h * 64 : (h + 1) * 64, pos : pos + 128],
                                on[:, i * 128 : (i + 1) * 128],
                            )

                # ---- reshard this batch's head-outputs to token-tiles ----
                nc.gpsimd.collective_compute(
                    "AllToAll",
                    AOP.bypass,
                    replica_groups=[list(range(N_CORES))],
                    ins=[cc_ins[b][:].opt()],
                    outs=[cc_outs[b][:].opt()],
                )

                # ---- output projection for my 2 token tiles of this batch ----
                for pos in range(2):
                    acts = []
                    for ft in range(N_CORES):
                        at = actp.tile([128, 128], BF, name="at", tag="at")
                        nc.sync.dma_start(
                            at[:], cc_outs[b][ft, :, pos * 128 : (pos + 1) * 128]
                        )
                        acts.append(at)
                    row0 = (2 * b + pos) * 128
                    for nn in range(2):
                        ps = ps_mm.tile([128, 512], FP, name="ps_op", tag="mm")
                        for ft in range(N_CORES):
                            nc.tensor.matmul(
                                ps[:],
                                lhsT=acts[ft][:],
                                rhs=wo_sb[:, ft, nn * 512 : (nn + 1) * 512],
                                start=(ft == 0),
                                stop=(ft == N_CORES - 1),
                            )
                        oo = oop.tile([128, 512], FP, name="oo", tag="oo")
                        nc.vector.tensor_copy(out=oo[:], in_=ps[:])
                        nc.sync.dma_start(
                            out[row0 : row0 + 128, nn * 512 : (nn + 1) * 512], oo[:]
                        )

    nc.finalize()
    return nc


_NC_CACHE = None


def _get_nc():
    global _NC_CACHE
    if _NC_CACHE is None:
        _NC_CACHE = build_nc()
    return _NC_CACHE


def make_in_maps(x, Wqkv, bqkv, Wo):
    import ml_dtypes

    bf16 = ml_dtypes.bfloat16
    scale = HD ** -0.5
    xtn = np.ascontiguousarray(x.reshape(NT, D).T).astype(bf16)  # [D, NT]
    mask = (np.arange(896)[None, :] - 384 >= np.arange(128)[:, None]).astype(bf16)
    ident = np.eye(128, dtype=np.float32).astype(bf16)
    wo = np.ascontiguousarray(Wo).astype(bf16)
    in_maps = []
    for c in range(N_CORES):
        h0, h1 = 2 * c, 2 * c + 1
        wqk_c = np.stack(
            [
                np.concatenate(
                    [Wqkv[h][:, 0:64] * scale, Wqkv[h][:, 64:128]], axis=1
                )
                for h in (h0, h1)
            ]
        ).astype(bf16)
        wv_c = np.concatenate(
            [Wqkv[h0][:, 128:192], Wqkv[h1][:, 128:192]], axis=1
        ).astype(bf16)
        bqk_c = np.stack(
            [
                np.concatenate([bqkv[h][0:64] * scale, bqkv[h][64:128]])[:, None]
                for h in (h0, h1)
            ]
        ).astype(np.float32)
        bv_c = np.concatenate([bqkv[h0][128:192], bqkv[h1][128:192]])[:, None].astype(
            np.float32
        )
        in_maps.append(
            {
                "xt": xtn,
                "wqk": np.ascontiguousarray(wqk_c),
                "wv": np.ascontiguousarray(wv_c),
                "bqk": np.ascontiguousarray(bqk_c),
                "bv": np.ascontiguousarray(bv_c),
                "wo": wo,
                "mask": mask,
                "ident": ident,
            }
        )
    return in_maps


def run_cores(in_maps, trace=False, trace_kwargs=None):
    nc = _get_nc()
    kwargs = {}
    if trace:
        kwargs["trace"] = True
        if trace_kwargs:
            kwargs["trace_kwargs"] = trace_kwargs
    return bass_utils.run_bass_kernel_spmd(
        nc, in_maps, core_ids=list(range(N_CORES)), **kwargs
    )


def assemble(results, bo):
    """Reassemble core outputs (interleaved token-tile mapping) into [B,S,D]."""
    full = np.empty((NT, D), np.float32)
    for c in range(N_CORES):
        o = results[c]["out"]
        for b in range(B):
            for pos in range(2):
                t = c + 8 * pos  # token tile within batch
                dst = b * S + t * 128
                full[dst : dst + 128] = o[(2 * b + pos) * 128 : (2 * b + pos + 1) * 128]
    full += bo[None, :]
    return full.reshape(B, S, D)


def kernel(x, Wqkv, bqkv, Wo, bo):
    x = np.asarray(x, dtype=np.float32)
    Wqkv = np.asarray(Wqkv, dtype=np.float32)
    bqkv = np.asarray(bqkv, dtype=np.float32)
    Wo = np.asarray(Wo, dtype=np.float32)
    bo = np.asarray(bo, dtype=np.float32)

    in_maps = make_in_maps(x, Wqkv, bqkv, Wo)
    res = run_cores(in_maps)
    return assemble(res.results, bo)


# revision 15
# speedup vs baseline: 1.1919x; 1.1919x over previous
"""Multi-head causal attention (B=4, S=2048, D=1024, H=16, HD=64) on 8 TRN2 cores.

Strategy:
  - Head-parallel: core i computes heads {2i, 2i+1} for all tokens.
    Host pre-transposes x -> xT [D, B*S], folds the 1/sqrt(HD) scale into Wq,
    converts matmul inputs to bf16, and adds bo at the end.
  - On device per core and per batch: qT/kT/vT projections (weights
    stationary, xT moving), scores computed transposed [k, q] with the two
    heads packed via PE row tiling (K=64 each), exp on ACT, PV matmul with
    stationary [v | 1] so the softmax denominator lands in output row 64,
    normalize via fast reciprocal + gpsimd partition_broadcast.
  - One AllToAll per batch (tokens of that batch, interleaved token-tile ->
    rank mapping) reshards head-outputs feature-major; the output projection
    for those tokens runs overlapped with the next batch's attention.
"""

import sys

sys.path.insert(0, "/opt/trn_rl_repo")

import numpy as np

import concourse.bass as bass
import concourse.mybir as mybir
import concourse.tile as tile
from concourse import bacc, bass_utils

FP = mybir.dt.float32
BF = mybir.dt.bfloat16
AOP = mybir.AluOpType
AFT = mybir.ActivationFunctionType

B, S, D, H = 4, 2048, 1024, 16
HD = 64
N_CORES = 8
NT = B * S  # 8192 tokens
TOK_PER_CORE = NT // N_CORES  # 1024
KD = D // 128  # 8 contraction tiles for the projections


def build_nc():
    nc = bacc.Bacc(None, target_bir_lowering=False, debug=False, num_devices=N_CORES)

    xt = nc.dram_tensor("xt", [D, NT], BF, kind="ExternalInput")
    wqk = nc.dram_tensor("wqk", [2, D, 128], BF, kind="ExternalInput")
    wv = nc.dram_tensor("wv", [D, 128], BF, kind="ExternalInput")
    bqk = nc.dram_tensor("bqk", [2, 128, 1], FP, kind="ExternalInput")
    bvb = nc.dram_tensor("bv", [128, 1], FP, kind="ExternalInput")
    wo = nc.dram_tensor("wo", [D, D], BF, kind="ExternalInput")
    maskd = nc.dram_tensor("mask", [128, 896], BF, kind="ExternalInput")
    identd = nc.dram_tensor("ident", [128, 128], BF, kind="ExternalInput")
    out = nc.dram_tensor("out", [TOK_PER_CORE, D], FP, kind="ExternalOutput")

    with tile.TileContext(nc) as tc:
        with (
            tc.tile_pool(name="const", bufs=1) as const,
            tc.tile_pool(name="xtp", bufs=16) as xtp,
            tc.tile_pool(name="qkv", bufs=2) as qkv,
            tc.tile_pool(name="vnp", bufs=18) as vnp,
            tc.tile_pool(name="esp", bufs=3) as esp,
            tc.tile_pool(name="small", bufs=4) as small,
            tc.tile_pool(name="onp", bufs=4) as onp,
            tc.tile_pool(name="actp", bufs=10) as actp,
            tc.tile_pool(name="oop", bufs=3) as oop,
            tc.tile_pool(name="ps_mm", bufs=2, space="PSUM") as ps_mm,
            tc.tile_pool(name="ps_s", bufs=2, space="PSUM") as ps_s,
            tc.tile_pool(name="ps_o", bufs=1, space="PSUM") as ps_o,
            tc.tile_pool(name="dram", bufs=1, space="DRAM") as dram,
        ):
            cc_ins = [
                dram.tile([N_CORES, 128, 256], BF, name=f"cc_in{b}") for b in range(B)
            ]
            cc_outs = [
                dram.tile([N_CORES, 128, 256], BF, name=f"cc_out{b}") for b in range(B)
            ]

            # ---- resident constants ----
            mask_sb = const.tile([128, 896], BF, name="mask_sb")
            nc.sync.dma_start(mask_sb[:], maskd[:])
            ident_sb = const.tile([128, 128], BF, name="ident_sb")
            nc.sync.dma_start(ident_sb[:], identd[:])
            wqk_sb = const.tile([128, 2 * KD, 128], BF, name="wqk_sb")
            nc.sync.dma_start(
                wqk_sb[:],
                wqk.rearrange("h (ko p) m -> p (h ko) m", p=128),
            )
            wv_sb = const.tile([128, KD, 128], BF, name="wv_sb")
            nc.sync.dma_start(wv_sb[:], wv.rearrange("(ko p) m -> p ko m", p=128))
            bqk_sb = const.tile([128, 2], FP, name="bqk_sb")
            nc.sync.dma_start(bqk_sb[:], bqk.rearrange("h p one -> p (h one)"))
            bv_sb = const.tile([128, 1], FP, name="bv_sb")
            nc.sync.dma_start(bv_sb[:], bvb[:])
            wo_sb = const.tile([128, KD, D], BF, name="wo_sb")

            for b in range(B):
                # ---- QKV projection for this batch (both heads) ----
                qT = qkv.tile([128, S], BF, name="qT", tag="qT")  # h0 rows 0-63, h1 64-127
                kT = qkv.tile([128, S], BF, name="kT", tag="kT")
                vT = qkv.tile([128, S], BF, name="vT", tag="vT")
                for st in range(4):  # 512-token slabs
                    xts = []
                    for kd in range(KD):
                        xt_t = xtp.tile([128, 512], BF, name="xt_t", tag="xt")
                        nc.sync.dma_start(
                            xt_t[:],
                            xt[kd * 128 : (kd + 1) * 128, b * S + st * 512 : b * S + (st + 1) * 512],
                        )
                        xts.append(xt_t)
                    for h in range(2):
                        ps = ps_mm.tile([128, 512], FP, name="ps_qk", tag="mm")
                        for kd in range(KD):
                            nc.tensor.matmul(
                                ps[:],
                                lhsT=wqk_sb[:, h * KD + kd, :],
                                rhs=xts[kd][:],
                                start=(kd == 0),
                                stop=(kd == KD - 1),
                            )
                        nc.vector.tensor_scalar(
                            qT[h * 64 : h * 64 + 64, st * 512 : (st + 1) * 512],
                            ps[0:64, :],
                            bqk_sb[0:64, h : h + 1],
                            None,
                            AOP.add,
                        )
                        nc.vector.tensor_scalar(
                            kT[h * 64 : h * 64 + 64, st * 512 : (st + 1) * 512],
                            ps[64:128, :],
                            bqk_sb[64:128, h : h + 1],
                            None,
                            AOP.add,
                        )
                    ps = ps_mm.tile([128, 512], FP, name="ps_v", tag="mm")
                    for kd in range(KD):
                        nc.tensor.matmul(
                            ps[:],
                            lhsT=wv_sb[:, kd, :],
                            rhs=xts[kd][:],
                            start=(kd == 0),
                            stop=(kd == KD - 1),
                        )
                    nc.vector.tensor_scalar(
                        vT[:, st * 512 : (st + 1) * 512],
                        ps[:],
                        bv_sb[:, 0:1],
                        None,
                        AOP.add,
                    )

                if b == 0:
                    nc.scalar.dma_start(
                        wo_sb[:], wo.rearrange("(ko p) n -> p ko n", p=128)
                    )

                # ---- vT -> v natural [token, hd] with ones columns ----
                vn_tiles = []
                for kc in range(S // 128):
                    pst = ps_mm.tile([128, 128], BF, name="ps_t", tag="mm")
                    nc.tensor.transpose(pst[:], vT[:, kc * 128 : (kc + 1) * 128], ident_sb[:])
                    vn = vnp.tile([128, 130], BF, name="vn", tag="vn")
                    nc.vector.tensor_copy(out=vn[:, 0:64], in_=pst[:, 0:64])
                    nc.vector.tensor_copy(out=vn[:, 65:129], in_=pst[:, 64:128])
                    nc.vector.tensor_copy(out=vn[:, 64:65], in_=mask_sb[:, 895:896])
                    nc.vector.tensor_copy(out=vn[:, 129:130], in_=mask_sb[:, 895:896])
                    vn_tiles.append(vn)

                # ---- causal attention, scores transposed [k, q] ----
                for qi in range(4):  # 512-wide query tiles
                    po = [
                        ps_o.tile([65, 512], FP, name=f"po{h}", tag=f"o{h}")
                        for h in range(2)
                    ]
                    nki = 4 * (qi + 1)
                    for ki in range(nki):
                        pss = ps_s.tile([128, 1024], FP, name="ps_sc", tag="sc")
                        for h in range(2):
                            nc.tensor.matmul(
                                pss[:, h * 512 : (h + 1) * 512],
                                lhsT=kT[h * 64 : h * 64 + 64, ki * 128 : (ki + 1) * 128],
                                rhs=qT[h * 64 : h * 64 + 64, qi * 512 : (qi + 1) * 512],
                                start=True,
                                stop=True,
                                tile_position=(h * 64, 0),
                            )
                        es = esp.tile([128, 1024], BF, name="es", tag="es")
                        nc.scalar.activation(es[:], pss[:], AFT.Exp)
                        if ki >= 4 * qi:  # diagonal tile: multiplicative causal mask
                            j = ki - 4 * qi
                            for h in range(2):
                                nc.vector.tensor_tensor(
                                    es[:, h * 512 : (h + 1) * 512],
                                    es[:, h * 512 : (h + 1) * 512],
                                    mask_sb[:, 384 - 128 * j : 896 - 128 * j],
                                    AOP.mult,
                                )
                        for h in range(2):
                            nc.tensor.matmul(
                                po[h][:],
                                lhsT=vn_tiles[ki][:, h * 65 : (h + 1) * 65],
                                rhs=es[:, h * 512 : (h + 1) * 512],
                                start=(ki == 0),
                                stop=(ki == nki - 1),
                            )
                    # normalize and scatter into this batch's A2A send buffer
                    for h in range(2):
                        oc = small.tile([65, 512], FP, name="oc", tag="oc")
                        nc.vector.tensor_copy(out=oc[:], in_=po[h][:])
                        den = small.tile([1, 512], FP, name="den", tag="den")
                        nc.vector.tensor_copy(out=den[:], in_=oc[64:65, :])
                        bc = small.tile([64, 512], FP, name="bc", tag="bc")
                        nc.gpsimd.partition_broadcast(bc[:], den[0:1, :], channels=64)
                        rc = small.tile([64, 512], FP, name="rc", tag="rc")
                        nc.vector.reciprocal_approx_fast(out=rc[:], in_=bc[:])
                        on = onp.tile([64, 512], BF, name="on", tag="on")
                        nc.vector.tensor_tensor(on[:], oc[0:64, :], rc[:], AOP.mult)
                        for i in range(4):
                            t = 4 * qi + i  # token tile within batch (0..15)
                            rank = t % 8
                            pos = 128 * (t // 8)
                            nc.sync.dma_start(
                                cc_ins[b][rank, h * 64 : (h + 1) * 64, pos : pos + 128],
                                on[:, i * 128 : (i + 1) * 128],
                            )

                # ---- reshard this batch's head-outputs to token-tiles ----
                nc.gpsimd.collective_compute(
                    "AllToAll",
                    AOP.bypass,
                    replica_groups=[list(range(N_CORES))],
                    ins=[cc_ins[b][:].opt()],
                    outs=[cc_outs[b][:].opt()],
                )

                # ---- output projection for my 2 token tiles of this batch ----
                for pos in range(2):
                    acts = []
                    for ft in range(N_CORES):
                        at = actp.tile([128, 128], BF, name="at", tag="at")
                        nc.sync.dma_start(
                            at[:], cc_outs[b][ft, :, pos * 128 : (pos + 1) * 128]
                        )
                        acts.append(at)
                    row0 = (2 * b + pos) * 128
                    for nn in range(2):
                        ps = ps_o.tile([128, 512], FP, name="ps_op", tag=f"o{nn}")
                        for ft in range(N_CORES):
                            nc.tensor.matmul(
                                ps[:],
                                lhsT=acts[ft][:],
                                rhs=wo_sb[:, ft, nn * 512 : (nn + 1) * 512],
                                start=(ft == 0),
                                stop=(ft == N_CORES - 1),
                            )
                        oo = oop.tile([128, 512], FP, name="oo", tag="oo")
                        nc.vector.tensor_copy(out=oo[:], in_=ps[:])
                        nc.sync.dma_start(
                            out[row0 : row0 + 128, nn * 512 : (nn + 1) * 512], oo[:]
                        )

    nc.finalize()
    return nc


_NC_CACHE = None


def _get_nc():
    global _NC_CACHE
    if _NC_CACHE is None:
        _NC_CACHE = build_nc()
    return _NC_CACHE


def make_in_maps(x, Wqkv, bqkv, Wo):
    import ml_dtypes

    bf16 = ml_dtypes.bfloat16
    scale = HD ** -0.5
    xtn = np.ascontiguousarray(x.reshape(NT, D).T).astype(bf16)  # [D, NT]
    mask = (np.arange(896)[None, :] - 384 >= np.arange(128)[:, None]).astype(bf16)
    ident = np.eye(128, dtype=np.float32).astype(bf16)
    wo = np.ascontiguousarray(Wo).astype(bf16)
    in_maps = []
    for c in range(N_CORES):
        h0, h1 = 2 * c, 2 * c + 1
        wqk_c = np.stack(
            [
                np.concatenate(
                    [Wqkv[h][:, 0:64] * scale, Wqkv[h][:, 64:128]], axis=1
                )
                for h in (h0, h1)
            ]
        ).astype(bf16)
        wv_c = np.concatenate(
            [Wqkv[h0][:, 128:192], Wqkv[h1][:, 128:192]], axis=1
        ).astype(bf16)
        bqk_c = np.stack(
            [
                np.concatenate([bqkv[h][0:64] * scale, bqkv[h][64:128]])[:, None]
                for h in (h0, h1)
            ]
        ).astype(np.float32)
        bv_c = np.concatenate([bqkv[h0][128:192], bqkv[h1][128:192]])[:, None].astype(
            np.float32
        )
        in_maps.append(
            {
                "xt": xtn,
                "wqk": np.ascontiguousarray(wqk_c),
                "wv": np.ascontiguousarray(wv_c),
                "bqk": np.ascontiguousarray(bqk_c),
                "bv": np.ascontiguousarray(bv_c),
                "wo": wo,
                "mask": mask,
                "ident": ident,
            }
        )
    return in_maps


def run_cores(in_maps, trace=False, trace_kwargs=None):
    nc = _get_nc()
    kwargs = {}
    if trace:
        kwargs["trace"] = True
        if trace_kwargs:
            kwargs["trace_kwargs"] = trace_kwargs
    return bass_utils.run_bass_kernel_spmd(
        nc, in_maps, core_ids=list(range(N_CORES)), **kwargs
    )


def assemble(results, bo):
    """Reassemble core outputs (interleaved token-tile mapping) into [B,S,D]."""
    full = np.empty((NT, D), np.float32)
    for c in range(N_CORES):
        o = results[c]["out"]
        for b in range(B):
            for pos in range(2):
                t = c + 8 * pos  # token tile within batch
                dst = b * S + t * 128
                full[dst : dst + 128] = o[(2 * b + pos) * 128 : (2 * b + pos + 1) * 128]
    full += bo[None, :]
    return full.reshape(B, S, D)


def kernel(x, Wqkv, bqkv, Wo, bo):
    x = np.asarray(x, dtype=np.float32)
    Wqkv = np.asarray(Wqkv, dtype=np.float32)
    bqkv = np.asarray(bqkv, dtype=np.float32)
    Wo = np.asarray(Wo, dtype=np.float32)
    bo = np.asarray(bo, dtype=np.float32)

    in_maps = make_in_maps(x, Wqkv, bqkv, Wo)
    res = run_cores(in_maps)
    return assemble(res.results, bo)


# revision 16
# speedup vs baseline: 1.3325x; 1.1180x over previous
"""Multi-head causal attention (B=4, S=2048, D=1024, H=16, HD=64) on 8 TRN2 cores.

Strategy:
  - Head-parallel: core i computes heads {2i, 2i+1} for all tokens.
    Host pre-transposes x -> xT [D, B*S], folds the 1/sqrt(HD) scale into Wq,
    converts matmul inputs to bf16, and adds bo at the end.
  - On device per core and per batch: qT/kT/vT projections (weights
    stationary, xT moving), scores computed transposed [k, q] with the two
    heads packed via PE row tiling (K=64 each), exp on ACT, PV matmul with
    stationary [v | 1] so the softmax denominator lands in output row 64,
    normalize via fast reciprocal + gpsimd partition_broadcast.
  - One AllToAll per batch (tokens of that batch, interleaved token-tile ->
    rank mapping) reshards head-outputs feature-major; the output projection
    for those tokens runs overlapped with the next batch's attention.
"""

import sys

sys.path.insert(0, "/opt/trn_rl_repo")

import numpy as np

import concourse.bass as bass
import concourse.mybir as mybir
import concourse.tile as tile
from concourse import bacc, bass_utils

FP = mybir.dt.float32
BF = mybir.dt.bfloat16
AOP = mybir.AluOpType
AFT = mybir.ActivationFunctionType

B, S, D, H = 4, 2048, 1024, 16
HD = 64
N_CORES = 8
NT = B * S  # 8192 tokens
TOK_PER_CORE = NT // N_CORES  # 1024
KD = D // 128  # 8 contraction tiles for the projections


def build_nc():
    nc = bacc.Bacc(None, target_bir_lowering=False, debug=False, num_devices=N_CORES)

    xt = nc.dram_tensor("xt", [D, NT], BF, kind="ExternalInput")
    wqk = nc.dram_tensor("wqk", [2, D, 128], BF, kind="ExternalInput")
    wv = nc.dram_tensor("wv", [D, 128], BF, kind="ExternalInput")
    bqk = nc.dram_tensor("bqk", [2, 128, 1], FP, kind="ExternalInput")
    bvb = nc.dram_tensor("bv", [128, 1], FP, kind="ExternalInput")
    wo = nc.dram_tensor("wo", [D, D], BF, kind="ExternalInput")
    maskd = nc.dram_tensor("mask", [128, 896], BF, kind="ExternalInput")
    identd = nc.dram_tensor("ident", [128, 128], BF, kind="ExternalInput")
    out = nc.dram_tensor("out", [TOK_PER_CORE, D], FP, kind="ExternalOutput")

    with tile.TileContext(nc) as tc:
        with (
            tc.tile_pool(name="const", bufs=1) as const,
            tc.tile_pool(name="xtp", bufs=16) as xtp,
            tc.tile_pool(name="qkv", bufs=2) as qkv,
            tc.tile_pool(name="vnp", bufs=18) as vnp,
            tc.tile_pool(name="esp", bufs=3) as esp,
            tc.tile_pool(name="small", bufs=4) as small,
            tc.tile_pool(name="onp", bufs=4) as onp,
            tc.tile_pool(name="actp", bufs=10) as actp,
            tc.tile_pool(name="oop", bufs=3) as oop,
            tc.tile_pool(name="ps_mm", bufs=2, space="PSUM") as ps_mm,
            tc.tile_pool(name="ps_s", bufs=2, space="PSUM") as ps_s,
            tc.tile_pool(name="ps_o", bufs=1, space="PSUM") as ps_o,
            tc.tile_pool(name="dram", bufs=1, space="DRAM") as dram,
        ):
            cc_ins = [
                dram.tile([N_CORES, 128, 256], BF, name=f"cc_in{b}") for b in range(B)
            ]
            cc_outs = [
                dram.tile([N_CORES, 128, 256], BF, name=f"cc_out{b}") for b in range(B)
            ]

            # ---- resident constants ----
            mask_sb = const.tile([128, 896], BF, name="mask_sb")
            nc.sync.dma_start(mask_sb[:], maskd[:])
            ident_sb = const.tile([128, 128], BF, name="ident_sb")
            nc.sync.dma_start(ident_sb[:], identd[:])
            wqk_sb = const.tile([128, 2 * KD, 128], BF, name="wqk_sb")
            nc.sync.dma_start(
                wqk_sb[:],
                wqk.rearrange("h (ko p) m -> p (h ko) m", p=128),
            )
            wv_sb = const.tile([128, KD, 128], BF, name="wv_sb")
            nc.sync.dma_start(wv_sb[:], wv.rearrange("(ko p) m -> p ko m", p=128))
            bqk_sb = const.tile([128, 2], FP, name="bqk_sb")
            nc.sync.dma_start(bqk_sb[:], bqk.rearrange("h p one -> p (h one)"))
            bv_sb = const.tile([128, 1], FP, name="bv_sb")
            nc.sync.dma_start(bv_sb[:], bvb[:])
            wo_sb = const.tile([128, KD, D], BF, name="wo_sb")

            def emit_oproj(bb):
                # output projection for my 2 token tiles of batch bb
                for pos in range(2):
                    acts = []
                    for ft in range(N_CORES):
                        at = actp.tile([128, 128], BF, name="at", tag="at")
                        nc.sync.dma_start(
                            at[:], cc_outs[bb][ft, :, pos * 128 : (pos + 1) * 128]
                        )
                        acts.append(at)
                    row0 = (2 * bb + pos) * 128
                    for nn in range(2):
                        ps = ps_o.tile([128, 512], FP, name="ps_op", tag=f"o{nn}")
                        for ft in range(N_CORES):
                            nc.tensor.matmul(
                                ps[:],
                                lhsT=acts[ft][:],
                                rhs=wo_sb[:, ft, nn * 512 : (nn + 1) * 512],
                                start=(ft == 0),
                                stop=(ft == N_CORES - 1),
                            )
                        oo = oop.tile([128, 512], FP, name="oo", tag="oo")
                        nc.vector.tensor_copy(out=oo[:], in_=ps[:])
                        nc.sync.dma_start(
                            out[row0 : row0 + 128, nn * 512 : (nn + 1) * 512], oo[:]
                        )

            for b in range(B):
                # ---- QKV projection for this batch (both heads) ----
                qT = qkv.tile([128, S], BF, name="qT", tag="qT")  # h0 rows 0-63, h1 64-127
                kT = qkv.tile([128, S], BF, name="kT", tag="kT")
                vT = qkv.tile([128, S], BF, name="vT", tag="vT")
                for st in range(4):  # 512-token slabs
                    xts = []
                    for kd in range(KD):
                        xt_t = xtp.tile([128, 512], BF, name="xt_t", tag="xt")
                        nc.sync.dma_start(
                            xt_t[:],
                            xt[kd * 128 : (kd + 1) * 128, b * S + st * 512 : b * S + (st + 1) * 512],
                        )
                        xts.append(xt_t)
                    for h in range(2):
                        ps = ps_mm.tile([128, 512], FP, name="ps_qk", tag="mm")
                        for kd in range(KD):
                            nc.tensor.matmul(
                                ps[:],
                                lhsT=wqk_sb[:, h * KD + kd, :],
                                rhs=xts[kd][:],
                                start=(kd == 0),
                                stop=(kd == KD - 1),
                            )
                        nc.vector.tensor_scalar(
                            qT[h * 64 : h * 64 + 64, st * 512 : (st + 1) * 512],
                            ps[0:64, :],
                            bqk_sb[0:64, h : h + 1],
                            None,
                            AOP.add,
                        )
                        nc.vector.tensor_scalar(
                            kT[h * 64 : h * 64 + 64, st * 512 : (st + 1) * 512],
                            ps[64:128, :],
                            bqk_sb[64:128, h : h + 1],
                            None,
                            AOP.add,
                        )
                    ps = ps_mm.tile([128, 512], FP, name="ps_v", tag="mm")
                    for kd in range(KD):
                        nc.tensor.matmul(
                            ps[:],
                            lhsT=wv_sb[:, kd, :],
                            rhs=xts[kd][:],
                            start=(kd == 0),
                            stop=(kd == KD - 1),
                        )
                    nc.vector.tensor_scalar(
                        vT[:, st * 512 : (st + 1) * 512],
                        ps[:],
                        bv_sb[:, 0:1],
                        None,
                        AOP.add,
                    )

                if b == 0:
                    nc.scalar.dma_start(
                        wo_sb[:], wo.rearrange("(ko p) n -> p ko n", p=128)
                    )

                # ---- vT -> v natural [token, hd] with ones columns ----
                vn_tiles = []
                for kc in range(S // 128):
                    pst = ps_mm.tile([128, 128], BF, name="ps_t", tag="mm")
                    nc.tensor.transpose(pst[:], vT[:, kc * 128 : (kc + 1) * 128], ident_sb[:])
                    vn = vnp.tile([128, 130], BF, name="vn", tag="vn")
                    nc.vector.tensor_copy(out=vn[:, 0:64], in_=pst[:, 0:64])
                    nc.vector.tensor_copy(out=vn[:, 65:129], in_=pst[:, 64:128])
                    nc.vector.tensor_copy(out=vn[:, 64:65], in_=mask_sb[:, 895:896])
                    nc.vector.tensor_copy(out=vn[:, 129:130], in_=mask_sb[:, 895:896])
                    vn_tiles.append(vn)

                # ---- causal attention, scores transposed [k, q] ----
                for qi in range(4):  # 512-wide query tiles
                    po = [
                        ps_o.tile([65, 512], FP, name=f"po{h}", tag=f"o{h}")
                        for h in range(2)
                    ]
                    nki = 4 * (qi + 1)
                    for ki in range(nki):
                        pss = ps_s.tile([128, 1024], FP, name="ps_sc", tag="sc")
                        for h in range(2):
                            nc.tensor.matmul(
                                pss[:, h * 512 : (h + 1) * 512],
                                lhsT=kT[h * 64 : h * 64 + 64, ki * 128 : (ki + 1) * 128],
                                rhs=qT[h * 64 : h * 64 + 64, qi * 512 : (qi + 1) * 512],
                                start=True,
                                stop=True,
                                tile_position=(h * 64, 0),
                            )
                        es = esp.tile([128, 1024], BF, name="es", tag="es")
                        nc.scalar.activation(es[:], pss[:], AFT.Exp)
                        if ki >= 4 * qi:  # diagonal tile: multiplicative causal mask
                            j = ki - 4 * qi
                            for h in range(2):
                                nc.vector.tensor_tensor(
                                    es[:, h * 512 : (h + 1) * 512],
                                    es[:, h * 512 : (h + 1) * 512],
                                    mask_sb[:, 384 - 128 * j : 896 - 128 * j],
                                    AOP.mult,
                                )
                        for h in range(2):
                            nc.tensor.matmul(
                                po[h][:],
                                lhsT=vn_tiles[ki][:, h * 65 : (h + 1) * 65],
                                rhs=es[:, h * 512 : (h + 1) * 512],
                                start=(ki == 0),
                                stop=(ki == nki - 1),
                            )
                    # normalize and scatter into this batch's A2A send buffer
                    for h in range(2):
                        oc = small.tile([65, 512], FP, name="oc", tag="oc")
                        nc.vector.tensor_copy(out=oc[:], in_=po[h][:])
                        den = small.tile([1, 512], FP, name="den", tag="den")
                        nc.vector.tensor_copy(out=den[:], in_=oc[64:65, :])
                        bc = small.tile([64, 512], FP, name="bc", tag="bc")
                        nc.gpsimd.partition_broadcast(bc[:], den[0:1, :], channels=64)
                        rc = small.tile([64, 512], FP, name="rc", tag="rc")
                        nc.vector.reciprocal_approx_fast(out=rc[:], in_=bc[:])
                        on = onp.tile([64, 512], BF, name="on", tag="on")
                        nc.vector.tensor_tensor(on[:], oc[0:64, :], rc[:], AOP.mult)
                        for i in range(4):
                            t = 4 * qi + i  # token tile within batch (0..15)
                            rank = t % 8
                            pos = 128 * (t // 8)
                            nc.sync.dma_start(
                                cc_ins[b][rank, h * 64 : (h + 1) * 64, pos : pos + 128],
                                on[:, i * 128 : (i + 1) * 128],
                            )

                # ---- reshard this batch's head-outputs to token-tiles ----
                nc.gpsimd.collective_compute(
                    "AllToAll",
                    AOP.bypass,
                    replica_groups=[list(range(N_CORES))],
                    ins=[cc_ins[b][:].opt()],
                    outs=[cc_outs[b][:].opt()],
                )

                # ---- output projection, software-pipelined one batch behind ----
                if b >= 1:
                    emit_oproj(b - 1)
            emit_oproj(B - 1)

    nc.finalize()
    return nc


_NC_CACHE = None


def _get_nc():
    global _NC_CACHE
    if _NC_CACHE is None:
        _NC_CACHE = build_nc()
    return _NC_CACHE


def make_in_maps(x, Wqkv, bqkv, Wo):
    import ml_dtypes

    bf16 = ml_dtypes.bfloat16
    scale = HD ** -0.5
    xtn = np.ascontiguousarray(x.reshape(NT, D).T).astype(bf16)  # [D, NT]
    mask = (np.arange(896)[None, :] - 384 >= np.arange(128)[:, None]).astype(bf16)
    ident = np.eye(128, dtype=np.float32).astype(bf16)
    wo = np.ascontiguousarray(Wo).astype(bf16)
    in_maps = []
    for c in range(N_CORES):
        h0, h1 = 2 * c, 2 * c + 1
        wqk_c = np.stack(
            [
                np.concatenate(
                    [Wqkv[h][:, 0:64] * scale, Wqkv[h][:, 64:128]], axis=1
                )
                for h in (h0, h1)
            ]
        ).astype(bf16)
        wv_c = np.concatenate(
            [Wqkv[h0][:, 128:192], Wqkv[h1][:, 128:192]], axis=1
        ).astype(bf16)
        bqk_c = np.stack(
            [
                np.concatenate([bqkv[h][0:64] * scale, bqkv[h][64:128]])[:, None]
                for h in (h0, h1)
            ]
        ).astype(np.float32)
        bv_c = np.concatenate([bqkv[h0][128:192], bqkv[h1][128:192]])[:, None].astype(
            np.float32
        )
        in_maps.append(
            {
                "xt": xtn,
                "wqk": np.ascontiguousarray(wqk_c),
                "wv": np.ascontiguousarray(wv_c),
                "bqk": np.ascontiguousarray(bqk_c),
                "bv": np.ascontiguousarray(bv_c),
                "wo": wo,
                "mask": mask,
                "ident": ident,
            }
        )
    return in_maps


def run_cores(in_maps, trace=False, trace_kwargs=None):
    nc = _get_nc()
    kwargs = {}
    if trace:
        kwargs["trace"] = True
        if trace_kwargs:
            kwargs["trace_kwargs"] = trace_kwargs
    return bass_utils.run_bass_kernel_spmd(
        nc, in_maps, core_ids=list(range(N_CORES)), **kwargs
    )


def assemble(results, bo):
    """Reassemble core outputs (interleaved token-tile mapping) into [B,S,D]."""
    full = np.empty((NT, D), np.float32)
    for c in range(N_CORES):
        o = results[c]["out"]
        for b in range(B):
            for pos in range(2):
                t = c + 8 * pos  # token tile within batch
                dst = b * S + t * 128
                full[dst : dst + 128] = o[(2 * b + pos) * 128 : (2 * b + pos + 1) * 128]
    full += bo[None, :]
    return full.reshape(B, S, D)


def kernel(x, Wqkv, bqkv, Wo, bo):
    x = np.asarray(x, dtype=np.float32)
    Wqkv = np.asarray(Wqkv, dtype=np.float32)
    bqkv = np.asarray(bqkv, dtype=np.float32)
    Wo = np.asarray(Wo, dtype=np.float32)
    bo = np.asarray(bo, dtype=np.float32)

    in_maps = make_in_maps(x, Wqkv, bqkv, Wo)
    res = run_cores(in_maps)
    return assemble(res.results, bo)


# revision 17
# speedup vs baseline: 1.3609x; 1.0213x over previous
"""Multi-head causal attention (B=4, S=2048, D=1024, H=16, HD=64) on 8 TRN2 cores.

Strategy:
  - Head-parallel: core i computes heads {2i, 2i+1} for all tokens.
    Host pre-transposes x -> xT [D, B*S], folds the 1/sqrt(HD) scale into Wq,
    converts matmul inputs to bf16, and adds bo at the end.
  - On device per core and per batch: qT/kT/vT projections (weights
    stationary, xT moving), scores computed transposed [k, q] with the two
    heads packed via PE row tiling (K=64 each), exp on ACT, PV matmul with
    stationary [v | 1] so the softmax denominator lands in output row 64,
    normalize via fast reciprocal + gpsimd partition_broadcast.
  - One AllToAll per batch (tokens of that batch, interleaved token-tile ->
    rank mapping) reshards head-outputs feature-major; the output projection
    for those tokens runs overlapped with the next batch's attention.
"""

import sys

sys.path.insert(0, "/opt/trn_rl_repo")

import numpy as np

import concourse.bass as bass
import concourse.mybir as mybir
import concourse.tile as tile
from concourse import bacc, bass_utils

FP = mybir.dt.float32
BF = mybir.dt.bfloat16
AOP = mybir.AluOpType
AFT = mybir.ActivationFunctionType

B, S, D, H = 4, 2048, 1024, 16
HD = 64
N_CORES = 8
NT = B * S  # 8192 tokens
TOK_PER_CORE = NT // N_CORES  # 1024
KD = D // 128  # 8 contraction tiles for the projections


def build_nc():
    nc = bacc.Bacc(None, target_bir_lowering=False, debug=False, num_devices=N_CORES)

    xt = nc.dram_tensor("xt", [D, NT], BF, kind="ExternalInput")
    wqk = nc.dram_tensor("wqk", [2, D, 128], BF, kind="ExternalInput")
    wv = nc.dram_tensor("wv", [D, 128], BF, kind="ExternalInput")
    bqk = nc.dram_tensor("bqk", [2, 128, 1], FP, kind="ExternalInput")
    bvb = nc.dram_tensor("bv", [128, 1], FP, kind="ExternalInput")
    wo = nc.dram_tensor("wo", [D, D], BF, kind="ExternalInput")
    maskd = nc.dram_tensor("mask", [128, 896], BF, kind="ExternalInput")
    identd = nc.dram_tensor("ident", [128, 128], BF, kind="ExternalInput")
    out = nc.dram_tensor("out", [TOK_PER_CORE, D], FP, kind="ExternalOutput")

    with tile.TileContext(nc) as tc:
        with (
            tc.tile_pool(name="const", bufs=1) as const,
            tc.tile_pool(name="xtp", bufs=16) as xtp,
            tc.tile_pool(name="qkv", bufs=2) as qkv,
            tc.tile_pool(name="vnp", bufs=18) as vnp,
            tc.tile_pool(name="esp", bufs=3) as esp,
            tc.tile_pool(name="small", bufs=4) as small,
            tc.tile_pool(name="onp", bufs=4) as onp,
            tc.tile_pool(name="actp", bufs=10) as actp,
            tc.tile_pool(name="oop", bufs=3) as oop,
            tc.tile_pool(name="ps_mm", bufs=2, space="PSUM") as ps_mm,
            tc.tile_pool(name="ps_s", bufs=2, space="PSUM") as ps_s,
            tc.tile_pool(name="ps_o", bufs=1, space="PSUM") as ps_o,
            tc.tile_pool(name="dram", bufs=1, space="DRAM") as dram,
        ):
            cc_ins = [
                dram.tile([N_CORES, 128, 256], BF, name=f"cc_in{b}") for b in range(B)
            ]
            cc_outs = [
                dram.tile([N_CORES, 128, 256], BF, name=f"cc_out{b}") for b in range(B)
            ]

            # ---- resident constants ----
            mask_sb = const.tile([128, 896], BF, name="mask_sb")
            nc.sync.dma_start(mask_sb[:], maskd[:])
            ident_sb = const.tile([128, 128], BF, name="ident_sb")
            nc.sync.dma_start(ident_sb[:], identd[:])
            wqk_sb = const.tile([128, 2 * KD, 128], BF, name="wqk_sb")
            nc.sync.dma_start(
                wqk_sb[:],
                wqk.rearrange("h (ko p) m -> p (h ko) m", p=128),
            )
            wv_sb = const.tile([128, KD, 128], BF, name="wv_sb")
            nc.sync.dma_start(wv_sb[:], wv.rearrange("(ko p) m -> p ko m", p=128))
            bqk_sb = const.tile([128, 2], FP, name="bqk_sb")
            nc.sync.dma_start(bqk_sb[:], bqk.rearrange("h p one -> p (h one)"))
            bv_sb = const.tile([128, 1], FP, name="bv_sb")
            nc.sync.dma_start(bv_sb[:], bvb[:])
            wo_sb = const.tile([128, KD, D], BF, name="wo_sb")

            def emit_oproj(bb):
                # output projection for my 2 token tiles of batch bb
                for pos in range(2):
                    acts = []
                    for ft in range(N_CORES):
                        at = actp.tile([128, 128], BF, name="at", tag="at")
                        nc.sync.dma_start(
                            at[:], cc_outs[bb][ft, :, pos * 128 : (pos + 1) * 128]
                        )
                        acts.append(at)
                    row0 = (2 * bb + pos) * 128
                    for nn in range(2):
                        ps = ps_o.tile([128, 512], FP, name="ps_op", tag=f"o{nn}")
                        for ft in range(N_CORES):
                            nc.tensor.matmul(
                                ps[:],
                                lhsT=acts[ft][:],
                                rhs=wo_sb[:, ft, nn * 512 : (nn + 1) * 512],
                                start=(ft == 0),
                                stop=(ft == N_CORES - 1),
                            )
                        oo = oop.tile([128, 512], FP, name="oo", tag="oo")
                        nc.vector.tensor_copy(out=oo[:], in_=ps[:])
                        nc.sync.dma_start(
                            out[row0 : row0 + 128, nn * 512 : (nn + 1) * 512], oo[:]
                        )

            qkv_tiles = {}

            def alloc_qkv(b):
                qkv_tiles[b] = (
                    qkv.tile([128, S], BF, name="qT", tag="qT"),
                    qkv.tile([128, S], BF, name="kT", tag="kT"),
                    qkv.tile([128, S], BF, name="vT", tag="vT"),
                )

            def emit_proj_st(b, st):
                qT, kT, vT = qkv_tiles[b]
                xts = []
                for kd in range(KD):
                    xt_t = xtp.tile([128, 512], BF, name="xt_t", tag="xt")
                    nc.sync.dma_start(
                        xt_t[:],
                        xt[kd * 128 : (kd + 1) * 128, b * S + st * 512 : b * S + (st + 1) * 512],
                    )
                    xts.append(xt_t)
                for h in range(2):
                    ps = ps_mm.tile([128, 512], FP, name="ps_qk", tag="mm")
                    for kd in range(KD):
                        nc.tensor.matmul(
                            ps[:],
                            lhsT=wqk_sb[:, h * KD + kd, :],
                            rhs=xts[kd][:],
                            start=(kd == 0),
                            stop=(kd == KD - 1),
                        )
                    nc.vector.tensor_scalar(
                        qT[h * 64 : h * 64 + 64, st * 512 : (st + 1) * 512],
                        ps[0:64, :],
                        bqk_sb[0:64, h : h + 1],
                        None,
                        AOP.add,
                    )
                    nc.vector.tensor_scalar(
                        kT[h * 64 : h * 64 + 64, st * 512 : (st + 1) * 512],
                        ps[64:128, :],
                        bqk_sb[64:128, h : h + 1],
                        None,
                        AOP.add,
                    )
                ps = ps_mm.tile([128, 512], FP, name="ps_v", tag="mm")
                for kd in range(KD):
                    nc.tensor.matmul(
                        ps[:],
                        lhsT=wv_sb[:, kd, :],
                        rhs=xts[kd][:],
                        start=(kd == 0),
                        stop=(kd == KD - 1),
                    )
                nc.vector.tensor_scalar(
                    vT[:, st * 512 : (st + 1) * 512],
                    ps[:],
                    bv_sb[:, 0:1],
                    None,
                    AOP.add,
                )

            vn_tiles = {}

            def emit_vtrans(b):
                # vT -> v natural [token, hd] tiles with ones columns
                _, _, vT = qkv_tiles[b]
                vn_tiles[b] = []
                for kc in range(S // 128):
                    pst = ps_mm.tile([128, 128], BF, name="ps_t", tag="mm")
                    nc.tensor.transpose(pst[:], vT[:, kc * 128 : (kc + 1) * 128], ident_sb[:])
                    vn = vnp.tile([128, 130], BF, name="vn", tag="vn")
                    nc.vector.tensor_copy(out=vn[:, 0:64], in_=pst[:, 0:64])
                    nc.vector.tensor_copy(out=vn[:, 65:129], in_=pst[:, 64:128])
                    nc.vector.tensor_copy(out=vn[:, 64:65], in_=mask_sb[:, 895:896])
                    nc.vector.tensor_copy(out=vn[:, 129:130], in_=mask_sb[:, 895:896])
                    vn_tiles[b].append(vn)

            def emit_attn_qi(b, qi):
                qT, kT, _ = qkv_tiles[b]
                po = [
                    ps_o.tile([65, 512], FP, name=f"po{h}", tag=f"o{h}")
                    for h in range(2)
                ]
                nki = 4 * (qi + 1)
                for ki in range(nki):
                    j = ki - 4 * qi  # >= 0 on diagonal tiles
                    c0 = 128 * max(j, 0)  # first useful column of this q-tile
                    pss = ps_s.tile([128, 1024], FP, name="ps_sc", tag="sc")
                    for h in range(2):
                        nc.tensor.matmul(
                            pss[:, h * 512 + c0 : (h + 1) * 512],
                            lhsT=kT[h * 64 : h * 64 + 64, ki * 128 : (ki + 1) * 128],
                            rhs=qT[h * 64 : h * 64 + 64, qi * 512 + c0 : (qi + 1) * 512],
                            start=True,
                            stop=True,
                            tile_position=(h * 64, 0),
                        )
                    es = esp.tile([128, 1024], BF, name="es", tag="es")
                    if c0 >= 256:
                        for h in range(2):
                            nc.scalar.activation(
                                es[:, h * 512 + c0 : (h + 1) * 512],
                                pss[:, h * 512 + c0 : (h + 1) * 512],
                                AFT.Exp,
                            )
                    else:
                        nc.scalar.activation(es[:], pss[:], AFT.Exp)
                    if j >= 0:  # diagonal tile: multiplicative causal mask
                        for h in range(2):
                            nc.vector.tensor_tensor(
                                es[:, h * 512 + c0 : (h + 1) * 512],
                                es[:, h * 512 + c0 : (h + 1) * 512],
                                mask_sb[:, 384 : 896 - c0],
                                AOP.mult,
                            )
                    for h in range(2):
                        nc.tensor.matmul(
                            po[h][:, c0:512],
                            lhsT=vn_tiles[b][ki][:, h * 65 : (h + 1) * 65],
                            rhs=es[:, h * 512 + c0 : (h + 1) * 512],
                            start=(ki == 0),
                            stop=(ki == nki - 1),
                        )
                # normalize and scatter into this batch's A2A send buffer
                for h in range(2):
                    oc = small.tile([65, 512], FP, name="oc", tag="oc")
                    nc.vector.tensor_copy(out=oc[:], in_=po[h][:])
                    den = small.tile([1, 512], FP, name="den", tag="den")
                    nc.vector.tensor_copy(out=den[:], in_=oc[64:65, :])
                    bc = small.tile([64, 512], FP, name="bc", tag="bc")
                    nc.gpsimd.partition_broadcast(bc[:], den[0:1, :], channels=64)
                    rc = small.tile([64, 512], FP, name="rc", tag="rc")
                    nc.vector.reciprocal_approx_fast(out=rc[:], in_=bc[:])
                    on = onp.tile([64, 512], BF, name="on", tag="on")
                    nc.vector.tensor_tensor(on[:], oc[0:64, :], rc[:], AOP.mult)
                    for i in range(4):
                        t = 4 * qi + i  # token tile within batch (0..15)
                        rank = t % 8
                        pos = 128 * (t // 8)
                        nc.sync.dma_start(
                            cc_ins[b][rank, h * 64 : (h + 1) * 64, pos : pos + 128],
                            on[:, i * 128 : (i + 1) * 128],
                        )

            # ---- software-pipelined schedule ----
            # prologue: batch 0 projections
            alloc_qkv(0)
            for st in range(4):
                emit_proj_st(0, st)
            nc.scalar.dma_start(wo_sb[:], wo.rearrange("(ko p) n -> p ko n", p=128))
            emit_vtrans(0)
            for b in range(B):
                if b + 1 < B:
                    alloc_qkv(b + 1)
                for qi in range(4):
                    emit_attn_qi(b, qi)
                    # interleave next batch's projection work into the
                    # ACT-gated attention stream
                    if b + 1 < B:
                        emit_proj_st(b + 1, qi)
                if b + 1 < B:
                    emit_vtrans(b + 1)
                # reshard this batch's head-outputs to token-tiles
                nc.gpsimd.collective_compute(
                    "AllToAll",
                    AOP.bypass,
                    replica_groups=[list(range(N_CORES))],
                    ins=[cc_ins[b][:].opt()],
                    outs=[cc_outs[b][:].opt()],
                )
                if b >= 1:
                    emit_oproj(b - 1)
                if b == 0:
                    del qkv_tiles[0]
            emit_oproj(B - 1)

    nc.finalize()
    return nc


_NC_CACHE = None


def _get_nc():
    global _NC_CACHE
    if _NC_CACHE is None:
        _NC_CACHE = build_nc()
    return _NC_CACHE


def make_in_maps(x, Wqkv, bqkv, Wo):
    import ml_dtypes

    bf16 = ml_dtypes.bfloat16
    scale = HD ** -0.5
    xtn = np.ascontiguousarray(x.reshape(NT, D).T).astype(bf16)  # [D, NT]
    mask = (np.arange(896)[None, :] - 384 >= np.arange(128)[:, None]).astype(bf16)
    ident = np.eye(128, dtype=np.float32).astype(bf16)
    wo = np.ascontiguousarray(Wo).astype(bf16)
    in_maps = []
    for c in range(N_CORES):
        h0, h1 = 2 * c, 2 * c + 1
        wqk_c = np.stack(
            [
                np.concatenate(
                    [Wqkv[h][:, 0:64] * scale, Wqkv[h][:, 64:128]], axis=1
                )
                for h in (h0, h1)
            ]
        ).astype(bf16)
        wv_c = np.concatenate(
            [Wqkv[h0][:, 128:192], Wqkv[h1][:, 128:192]], axis=1
        ).astype(bf16)
        bqk_c = np.stack(
            [
                np.concatenate([bqkv[h][0:64] * scale, bqkv[h][64:128]])[:, None]
                for h in (h0, h1)
            ]
        ).astype(np.float32)
        bv_c = np.concatenate([bqkv[h0][128:192], bqkv[h1][128:192]])[:, None].astype(
            np.float32
        )
        in_maps.append(
            {
                "xt": xtn,
                "wqk": np.ascontiguousarray(wqk_c),
                "wv": np.ascontiguousarray(wv_c),
                "bqk": np.ascontiguousarray(bqk_c),
                "bv": np.ascontiguousarray(bv_c),
                "wo": wo,
                "mask": mask,
                "ident": ident,
            }
        )
    return in_maps


def run_cores(in_maps, trace=False, trace_kwargs=None):
    nc = _get_nc()
    kwargs = {}
    if trace:
        kwargs["trace"] = True
        if trace_kwargs:
            kwargs["trace_kwargs"] = trace_kwargs
    return bass_utils.run_bass_kernel_spmd(
        nc, in_maps, core_ids=list(range(N_CORES)), **kwargs
    )


def assemble(results, bo):
    """Reassemble core outputs (interleaved token-tile mapping) into [B,S,D]."""
    full = np.empty((NT, D), np.float32)
    for c in range(N_CORES):
        o = results[c]["out"]
        for b in range(B):
            for pos in range(2):
                t = c + 8 * pos  # token tile within batch
                dst = b * S + t * 128
                full[dst : dst + 128] = o[(2 * b + pos) * 128 : (2 * b + pos + 1) * 128]
    full += bo[None, :]
    return full.reshape(B, S, D)


def kernel(x, Wqkv, bqkv, Wo, bo):
    x = np.asarray(x, dtype=np.float32)
    Wqkv = np.asarray(Wqkv, dtype=np.float32)
    bqkv = np.asarray(bqkv, dtype=np.float32)
    Wo = np.asarray(Wo, dtype=np.float32)
    bo = np.asarray(bo, dtype=np.float32)

    in_maps = make_in_maps(x, Wqkv, bqkv, Wo)
    res = run_cores(in_maps)
    return assemble(res.results, bo)


# revision 18
# speedup vs baseline: 1.3739x; 1.0095x over previous
"""Multi-head causal attention (B=4, S=2048, D=1024, H=16, HD=64) on 8 TRN2 cores.

Strategy:
  - Head-parallel: core i computes heads {2i, 2i+1} for all tokens.
    Host pre-transposes x -> xT [D, B*S], folds the 1/sqrt(HD) scale into Wq,
    converts matmul inputs to bf16, and adds bo at the end.
  - On device per core and per batch: qT/kT/vT projections (weights
    stationary, xT moving), scores computed transposed [k, q] with the two
    heads packed via PE row tiling (K=64 each), exp on ACT, PV matmul with
    stationary [v | 1] so the softmax denominator lands in output row 64,
    normalize via fast reciprocal + gpsimd partition_broadcast.
  - One AllToAll per batch (tokens of that batch, interleaved token-tile ->
    rank mapping) reshards head-outputs feature-major; the output projection
    for those tokens runs overlapped with the next batch's attention.
"""

import sys

sys.path.insert(0, "/opt/trn_rl_repo")

import numpy as np

import concourse.bass as bass
import concourse.mybir as mybir
import concourse.tile as tile
from concourse import bacc, bass_utils

FP = mybir.dt.float32
BF = mybir.dt.bfloat16
AOP = mybir.AluOpType
AFT = mybir.ActivationFunctionType

B, S, D, H = 4, 2048, 1024, 16
HD = 64
N_CORES = 8
NT = B * S  # 8192 tokens
TOK_PER_CORE = NT // N_CORES  # 1024
KD = D // 128  # 8 contraction tiles for the projections


def build_nc():
    nc = bacc.Bacc(None, target_bir_lowering=False, debug=False, num_devices=N_CORES)

    xt = nc.dram_tensor("xt", [D, NT], BF, kind="ExternalInput")
    wqk = nc.dram_tensor("wqk", [2, D, 128], BF, kind="ExternalInput")
    wv = nc.dram_tensor("wv", [D, 128], BF, kind="ExternalInput")
    bqk = nc.dram_tensor("bqk", [2, 128, 1], FP, kind="ExternalInput")
    bvb = nc.dram_tensor("bv", [128, 1], FP, kind="ExternalInput")
    wo = nc.dram_tensor("wo", [D, D], BF, kind="ExternalInput")
    maskd = nc.dram_tensor("mask", [128, 896], BF, kind="ExternalInput")
    identd = nc.dram_tensor("ident", [128, 128], BF, kind="ExternalInput")
    out = nc.dram_tensor("out", [TOK_PER_CORE, D], FP, kind="ExternalOutput")

    with tile.TileContext(nc) as tc:
        with (
            tc.tile_pool(name="const", bufs=1) as const,
            tc.tile_pool(name="xtp", bufs=20) as xtp,
            tc.tile_pool(name="qkv", bufs=2) as qkv,
            tc.tile_pool(name="vnp", bufs=18) as vnp,
            tc.tile_pool(name="esp", bufs=4) as esp,
            tc.tile_pool(name="small", bufs=4) as small,
            tc.tile_pool(name="onp", bufs=6) as onp,
            tc.tile_pool(name="actp", bufs=16) as actp,
            tc.tile_pool(name="oop", bufs=3) as oop,
            tc.tile_pool(name="ps_mm", bufs=2, space="PSUM") as ps_mm,
            tc.tile_pool(name="ps_s", bufs=2, space="PSUM") as ps_s,
            tc.tile_pool(name="ps_o", bufs=1, space="PSUM") as ps_o,
            tc.tile_pool(name="dram", bufs=1, space="DRAM") as dram,
        ):
            cc_ins = [
                [
                    dram.tile([N_CORES, 128, 128], BF, name=f"cc_in{b}_{hf}")
                    for hf in range(2)
                ]
                for b in range(B)
            ]
            cc_outs = [
                [
                    dram.tile([N_CORES, 128, 128], BF, name=f"cc_out{b}_{hf}")
                    for hf in range(2)
                ]
                for b in range(B)
            ]

            # ---- resident constants ----
            mask_sb = const.tile([128, 896], BF, name="mask_sb")
            nc.sync.dma_start(mask_sb[:], maskd[:])
            ident_sb = const.tile([128, 128], BF, name="ident_sb")
            nc.sync.dma_start(ident_sb[:], identd[:])
            wqk_sb = const.tile([128, 2 * KD, 128], BF, name="wqk_sb")
            wqk_r = wqk.rearrange("h (ko p) m -> p (h ko) m", p=128)
            for kd in range(2 * KD):
                nc.sync.dma_start(wqk_sb[:, kd, :], wqk_r[:, kd, :])
            wv_sb = const.tile([128, KD, 128], BF, name="wv_sb")
            nc.sync.dma_start(wv_sb[:], wv.rearrange("(ko p) m -> p ko m", p=128))
            bqk_sb = const.tile([128, 2], FP, name="bqk_sb")
            nc.sync.dma_start(bqk_sb[:], bqk.rearrange("h p one -> p (h one)"))
            bv_sb = const.tile([128, 1], FP, name="bv_sb")
            nc.sync.dma_start(bv_sb[:], bvb[:])
            wo_sb = const.tile([128, KD, D], BF, name="wo_sb")

            def emit_oproj(bb, pos):
                # output projection for my token tile (batch bb, half pos)
                acts = []
                for ft in range(N_CORES):
                    at = actp.tile([128, 128], BF, name="at", tag="at")
                    nc.sync.dma_start(at[:], cc_outs[bb][pos][ft, :, :])
                    acts.append(at)
                row0 = (2 * bb + pos) * 128
                for nn in range(2):
                    ps = ps_o.tile([128, 512], FP, name="ps_op", tag=f"o{nn}")
                    for ft in range(N_CORES):
                        nc.tensor.matmul(
                            ps[:],
                            lhsT=acts[ft][:],
                            rhs=wo_sb[:, ft, nn * 512 : (nn + 1) * 512],
                            start=(ft == 0),
                            stop=(ft == N_CORES - 1),
                        )
                    oo = oop.tile([128, 512], FP, name="oo", tag="oo")
                    nc.vector.tensor_copy(out=oo[:], in_=ps[:])
                    nc.sync.dma_start(
                        out[row0 : row0 + 128, nn * 512 : (nn + 1) * 512], oo[:]
                    )

            def emit_a2a(bb, hf):
                nc.gpsimd.collective_compute(
                    "AllToAll",
                    AOP.bypass,
                    replica_groups=[list(range(N_CORES))],
                    ins=[cc_ins[bb][hf][:].opt()],
                    outs=[cc_outs[bb][hf][:].opt()],
                )

            qkv_tiles = {}

            def alloc_qkv(b):
                qkv_tiles[b] = (
                    qkv.tile([128, S], BF, name="qT", tag="qT"),
                    qkv.tile([128, S], BF, name="kT", tag="kT"),
                    qkv.tile([128, S], BF, name="vT", tag="vT"),
                )

            def emit_proj_st(b, st):
                qT, kT, vT = qkv_tiles[b]
                xts = []
                for kd in range(KD):
                    xt_t = xtp.tile([128, 512], BF, name="xt_t", tag="xt")
                    nc.sync.dma_start(
                        xt_t[:],
                        xt[kd * 128 : (kd + 1) * 128, b * S + st * 512 : b * S + (st + 1) * 512],
                    )
                    xts.append(xt_t)
                for h in range(2):
                    ps = ps_mm.tile([128, 512], FP, name="ps_qk", tag="mm")
                    for kd in range(KD):
                        nc.tensor.matmul(
                            ps[:],
                            lhsT=wqk_sb[:, h * KD + kd, :],
                            rhs=xts[kd][:],
                            start=(kd == 0),
                            stop=(kd == KD - 1),
                        )
                    nc.vector.tensor_scalar(
                        qT[h * 64 : h * 64 + 64, st * 512 : (st + 1) * 512],
                        ps[0:64, :],
                        bqk_sb[0:64, h : h + 1],
                        None,
                        AOP.add,
                    )
                    nc.vector.tensor_scalar(
                        kT[h * 64 : h * 64 + 64, st * 512 : (st + 1) * 512],
                        ps[64:128, :],
                        bqk_sb[64:128, h : h + 1],
                        None,
                        AOP.add,
                    )
                ps = ps_mm.tile([128, 512], FP, name="ps_v", tag="mm")
                for kd in range(KD):
                    nc.tensor.matmul(
                        ps[:],
                        lhsT=wv_sb[:, kd, :],
                        rhs=xts[kd][:],
                        start=(kd == 0),
                        stop=(kd == KD - 1),
                    )
                nc.vector.tensor_scalar(
                    vT[:, st * 512 : (st + 1) * 512],
                    ps[:],
                    bv_sb[:, 0:1],
                    None,
                    AOP.add,
                )

            vn_tiles = {}

            def emit_vtrans(b):
                # vT -> v natural [token, hd] tiles with ones columns
                _, _, vT = qkv_tiles[b]
                vn_tiles[b] = []
                for kc in range(S // 128):
                    pst = ps_mm.tile([128, 128], BF, name="ps_t", tag="mm")
                    nc.tensor.transpose(pst[:], vT[:, kc * 128 : (kc + 1) * 128], ident_sb[:])
                    vn = vnp.tile([128, 130], BF, name="vn", tag="vn")
                    nc.vector.tensor_copy(out=vn[:, 0:64], in_=pst[:, 0:64])
                    nc.vector.tensor_copy(out=vn[:, 65:129], in_=pst[:, 64:128])
                    nc.vector.tensor_copy(out=vn[:, 64:65], in_=mask_sb[:, 895:896])
                    nc.vector.tensor_copy(out=vn[:, 129:130], in_=mask_sb[:, 895:896])
                    vn_tiles[b].append(vn)

            def emit_attn_qi(b, qi):
                qT, kT, _ = qkv_tiles[b]
                po = [
                    ps_o.tile([65, 512], FP, name=f"po{h}", tag=f"o{h}")
                    for h in range(2)
                ]
                nki = 4 * (qi + 1)
                for ki in range(nki):
                    j = ki - 4 * qi  # >= 0 on diagonal tiles
                    c0 = 128 * max(j, 0)  # first useful column of this q-tile
                    pss = ps_s.tile([128, 1024], FP, name="ps_sc", tag="sc")
                    for h in range(2):
                        nc.tensor.matmul(
                            pss[:, h * 512 + c0 : (h + 1) * 512],
                            lhsT=kT[h * 64 : h * 64 + 64, ki * 128 : (ki + 1) * 128],
                            rhs=qT[h * 64 : h * 64 + 64, qi * 512 + c0 : (qi + 1) * 512],
                            start=True,
                            stop=True,
                            tile_position=(h * 64, 0),
                        )
                    es = esp.tile([128, 1024], BF, name="es", tag="es")
                    if c0 >= 256:
                        for h in range(2):
                            nc.scalar.activation(
                                es[:, h * 512 + c0 : (h + 1) * 512],
                                pss[:, h * 512 + c0 : (h + 1) * 512],
                                AFT.Exp,
                            )
                    else:
                        nc.scalar.activation(es[:], pss[:], AFT.Exp)
                    if j >= 0:  # diagonal tile: multiplicative causal mask
                        for h in range(2):
                            nc.vector.tensor_tensor(
                                es[:, h * 512 + c0 : (h + 1) * 512],
                                es[:, h * 512 + c0 : (h + 1) * 512],
                                mask_sb[:, 384 : 896 - c0],
                                AOP.mult,
                            )
                    for h in range(2):
                        nc.tensor.matmul(
                            po[h][:, c0:512],
                            lhsT=vn_tiles[b][ki][:, h * 65 : (h + 1) * 65],
                            rhs=es[:, h * 512 + c0 : (h + 1) * 512],
                            start=(ki == 0),
                            stop=(ki == nki - 1),
                        )
                # normalize and scatter into this batch's A2A send buffer
                for h in range(2):
                    oc = small.tile([65, 512], FP, name="oc", tag="oc")
                    nc.vector.tensor_copy(out=oc[:], in_=po[h][:])
                    den = small.tile([1, 512], FP, name="den", tag="den")
                    nc.vector.tensor_copy(out=den[:], in_=oc[64:65, :])
                    bc = small.tile([64, 512], FP, name="bc", tag="bc")
                    nc.gpsimd.partition_broadcast(bc[:], den[0:1, :], channels=64)
                    rc = small.tile([64, 512], FP, name="rc", tag="rc")
                    nc.vector.reciprocal_approx_fast(out=rc[:], in_=bc[:])
                    on = onp.tile([64, 512], BF, name="on", tag="on")
                    nc.vector.tensor_tensor(on[:], oc[0:64, :], rc[:], AOP.mult)
                    for i in range(4):
                        t = 4 * qi + i  # token tile within batch (0..15)
                        nc.sync.dma_start(
                            cc_ins[b][t // 8][t % 8, h * 64 : (h + 1) * 64, :],
                            on[:, i * 128 : (i + 1) * 128],
                        )

            # ---- software-pipelined schedule ----
            # prologue: batch 0 projections
            alloc_qkv(0)
            for st in range(4):
                emit_proj_st(0, st)
            nc.scalar.dma_start(wo_sb[:], wo.rearrange("(ko p) n -> p ko n", p=128))
            emit_vtrans(0)
            for b in range(B):
                if b + 1 < B:
                    alloc_qkv(b + 1)
                for qi in range(4):
                    emit_attn_qi(b, qi)
                    if qi == 1:
                        emit_a2a(b, 0)
                    # interleave next batch's projection work into the
                    # ACT-gated attention stream
                    if b + 1 < B:
                        emit_proj_st(b + 1, qi)
                if b + 1 < B:
                    emit_vtrans(b + 1)
                emit_a2a(b, 1)
                if b >= 1:
                    emit_oproj(b - 1, 0)
                    emit_oproj(b - 1, 1)
            emit_oproj(B - 1, 0)
            emit_oproj(B - 1, 1)

    nc.finalize()
    return nc


_NC_CACHE = None


def _get_nc():
    global _NC_CACHE
    if _NC_CACHE is None:
        _NC_CACHE = build_nc()
    return _NC_CACHE


def make_in_maps(x, Wqkv, bqkv, Wo):
    import ml_dtypes

    bf16 = ml_dtypes.bfloat16
    scale = HD ** -0.5
    xtn = np.ascontiguousarray(x.reshape(NT, D).T).astype(bf16)  # [D, NT]
    mask = (np.arange(896)[None, :] - 384 >= np.arange(128)[:, None]).astype(bf16)
    ident = np.eye(128, dtype=np.float32).astype(bf16)
    wo = np.ascontiguousarray(Wo).astype(bf16)
    in_maps = []
    for c in range(N_CORES):
        h0, h1 = 2 * c, 2 * c + 1
        wqk_c = np.stack(
            [
                np.concatenate(
                    [Wqkv[h][:, 0:64] * scale, Wqkv[h][:, 64:128]], axis=1
                )
                for h in (h0, h1)
            ]
        ).astype(bf16)
        wv_c = np.concatenate(
            [Wqkv[h0][:, 128:192], Wqkv[h1][:, 128:192]], axis=1
        ).astype(bf16)
        bqk_c = np.stack(
            [
                np.concatenate([bqkv[h][0:64] * scale, bqkv[h][64:128]])[:, None]
                for h in (h0, h1)
            ]
        ).astype(np.float32)
        bv_c = np.concatenate([bqkv[h0][128:192], bqkv[h1][128:192]])[:, None].astype(
            np.float32
        )
        in_maps.append(
            {
                "xt": xtn,
                "wqk": np.ascontiguousarray(wqk_c),
                "wv": np.ascontiguousarray(wv_c),
                "bqk": np.ascontiguousarray(bqk_c),
                "bv": np.ascontiguousarray(bv_c),
                "wo": wo,
                "mask": mask,
                "ident": ident,
            }
        )
    return in_maps


def run_cores(in_maps, trace=False, trace_kwargs=None):
    nc = _get_nc()
    kwargs = {}
    if trace:
        kwargs["trace"] = True
        if trace_kwargs:
            kwargs["trace_kwargs"] = trace_kwargs
    return bass_utils.run_bass_kernel_spmd(
        nc, in_maps, core_ids=list(range(N_CORES)), **kwargs
    )


def assemble(results, bo):
    """Reassemble core outputs (interleaved token-tile mapping) into [B,S,D]."""
    full = np.empty((NT, D), np.float32)
    for c in range(N_CORES):
        o = results[c]["out"]
        for b in range(B):
            for pos in range(2):
                t = c + 8 * pos  # token tile within batch
                dst = b * S + t * 128
                full[dst : dst + 128] = o[(2 * b + pos) * 128 : (2 * b + pos + 1) * 128]
    full += bo[None, :]
    return full.reshape(B, S, D)


def kernel(x, Wqkv, bqkv, Wo, bo):
    x = np.asarray(x, dtype=np.float32)
    Wqkv = np.asarray(Wqkv, dtype=np.float32)
    bqkv = np.asarray(bqkv, dtype=np.float32)
    Wo = np.asarray(Wo, dtype=np.float32)
    bo = np.asarray(bo, dtype=np.float32)

    in_maps = make_in_maps(x, Wqkv, bqkv, Wo)
    res = run_cores(in_maps)
    return assemble(res.results, bo)


# revision 19
# speedup vs baseline: 1.4651x; 1.0664x over previous
"""Multi-head causal attention (B=4, S=2048, D=1024, H=16, HD=64) on 8 TRN2 cores.

Strategy:
  - Head-parallel: core i computes heads {2i, 2i+1} for all tokens.
    Host pre-transposes x -> xT [D, B*S], folds the 1/sqrt(HD) scale into Wq,
    converts matmul inputs to bf16, and adds bo at the end.
  - On device per core and per batch: qT/kT/vT projections (weights
    stationary, xT moving), scores computed transposed [k, q] with the two
    heads packed via PE row tiling (K=64 each), exp on ACT, PV matmul with
    stationary [v | 1] so the softmax denominator lands in output row 64,
    normalize via fast reciprocal + gpsimd partition_broadcast.
  - One AllToAll per batch (tokens of that batch, interleaved token-tile ->
    rank mapping) reshards head-outputs feature-major; the output projection
    for those tokens runs overlapped with the next batch's attention.
"""

import sys

sys.path.insert(0, "/opt/trn_rl_repo")

import numpy as np

import concourse.bass as bass
import concourse.mybir as mybir
import concourse.tile as tile
from concourse import bacc, bass_utils

FP = mybir.dt.float32
BF = mybir.dt.bfloat16
AOP = mybir.AluOpType
AFT = mybir.ActivationFunctionType

B, S, D, H = 4, 2048, 1024, 16
HD = 64
N_CORES = 8
NT = B * S  # 8192 tokens
TOK_PER_CORE = NT // N_CORES  # 1024
KD = D // 128  # 8 contraction tiles for the projections


def build_nc():
    nc = bacc.Bacc(None, target_bir_lowering=False, debug=False, num_devices=N_CORES)

    xt = nc.dram_tensor("xt", [D, NT], BF, kind="ExternalInput")
    wqk = nc.dram_tensor("wqk", [2, D, 128], BF, kind="ExternalInput")
    wv = nc.dram_tensor("wv", [D, 128], BF, kind="ExternalInput")
    bqk = nc.dram_tensor("bqk", [2, 128, 1], FP, kind="ExternalInput")
    bvb = nc.dram_tensor("bv", [128, 1], FP, kind="ExternalInput")
    wo = nc.dram_tensor("wo", [D, D], BF, kind="ExternalInput")
    maskd = nc.dram_tensor("mask", [128, 896], BF, kind="ExternalInput")
    identd = nc.dram_tensor("ident", [128, 128], BF, kind="ExternalInput")
    out = nc.dram_tensor("out", [TOK_PER_CORE, D], FP, kind="ExternalOutput")

    with tile.TileContext(nc) as tc:
        with (
            tc.tile_pool(name="const", bufs=1) as const,
            tc.tile_pool(name="xtp", bufs=4) as xtp,
            tc.tile_pool(name="qkv", bufs=2) as qkv,
            tc.tile_pool(name="vnp", bufs=18) as vnp,
            tc.tile_pool(name="esp", bufs=4) as esp,
            tc.tile_pool(name="small", bufs=4) as small,
            tc.tile_pool(name="onp", bufs=6) as onp,
            tc.tile_pool(name="actp", bufs=3) as actp,
            tc.tile_pool(name="oop", bufs=3) as oop,
            tc.tile_pool(name="ps_mm", bufs=2, space="PSUM") as ps_mm,
            tc.tile_pool(name="ps_s", bufs=2, space="PSUM") as ps_s,
            tc.tile_pool(name="ps_o", bufs=1, space="PSUM") as ps_o,
            tc.tile_pool(name="dram", bufs=1, space="DRAM") as dram,
        ):
            cc_ins = [
                [
                    dram.tile([N_CORES, 128, 128], BF, name=f"cc_in{b}_{hf}")
                    for hf in range(2)
                ]
                for b in range(B)
            ]
            cc_outs = [
                [
                    dram.tile([N_CORES, 128, 128], BF, name=f"cc_out{b}_{hf}")
                    for hf in range(2)
                ]
                for b in range(B)
            ]

            # ---- resident constants ----
            wqk_sb = const.tile([128, 2 * KD, 128], BF, name="wqk_sb")
            nc.scalar.dma_start(
                wqk_sb[:], wqk.rearrange("h (ko p) m -> p (h ko) m", p=128)
            )
            wv_sb = const.tile([128, KD, 128], BF, name="wv_sb")
            nc.scalar.dma_start(wv_sb[:], wv.rearrange("(ko p) m -> p ko m", p=128))
            mask_sb = const.tile([128, 896], BF, name="mask_sb")
            nc.scalar.dma_start(mask_sb[:], maskd[:])
            ident_sb = const.tile([128, 128], BF, name="ident_sb")
            nc.scalar.dma_start(ident_sb[:], identd[:])
            bqk_sb = const.tile([128, 2], FP, name="bqk_sb")
            nc.scalar.dma_start(bqk_sb[:], bqk.rearrange("h p one -> p (h one)"))
            bv_sb = const.tile([128, 1], FP, name="bv_sb")
            nc.scalar.dma_start(bv_sb[:], bvb[:])
            wo_sb = const.tile([128, KD, D], BF, name="wo_sb")

            def emit_oproj(bb, pos):
                # output projection for my token tile (batch bb, half pos)
                at = actp.tile([128, N_CORES, 128], BF, name="at", tag="at")
                nc.sync.dma_start(at[:], cc_outs[bb][pos][:].rearrange("f p t -> p f t"))
                acts = [at[:, ft, :] for ft in range(N_CORES)]
                row0 = (2 * bb + pos) * 128
                for nn in range(2):
                    ps = ps_o.tile([128, 512], FP, name="ps_op", tag=f"o{nn}")
                    for ft in range(N_CORES):
                        nc.tensor.matmul(
                            ps[:],
                            lhsT=acts[ft],
                            rhs=wo_sb[:, ft, nn * 512 : (nn + 1) * 512],
                            start=(ft == 0),
                            stop=(ft == N_CORES - 1),
                        )
                    oo = oop.tile([128, 512], FP, name="oo", tag="oo")
                    nc.vector.tensor_copy(out=oo[:], in_=ps[:])
                    nc.sync.dma_start(
                        out[row0 : row0 + 128, nn * 512 : (nn + 1) * 512], oo[:]
                    )

            def emit_a2a(bb, hf):
                nc.gpsimd.collective_compute(
                    "AllToAll",
                    AOP.bypass,
                    replica_groups=[list(range(N_CORES))],
                    ins=[cc_ins[bb][hf][:].opt()],
                    outs=[cc_outs[bb][hf][:].opt()],
                )

            qkv_tiles = {}

            def alloc_qkv(b):
                qkv_tiles[b] = (
                    qkv.tile([128, S], BF, name="qT", tag="qT"),
                    qkv.tile([128, S], BF, name="kT", tag="kT"),
                    qkv.tile([128, S], BF, name="vT", tag="vT"),
                )

            xt_r = xt.rearrange("(k p) n -> p k n", p=128)

            def emit_proj_st(b, st):
                qT, kT, vT = qkv_tiles[b]
                xt_st = xtp.tile([128, KD, 512], BF, name="xt_st", tag="xt")
                nc.sync.dma_start(
                    xt_st[:],
                    xt_r[:, :, b * S + st * 512 : b * S + (st + 1) * 512],
                )
                xts = [xt_st[:, kd, :] for kd in range(KD)]
                for h in range(2):
                    ps = ps_mm.tile([128, 512], FP, name="ps_qk", tag="mm")
                    for kd in range(KD):
                        nc.tensor.matmul(
                            ps[:],
                            lhsT=wqk_sb[:, h * KD + kd, :],
                            rhs=xts[kd],
                            start=(kd == 0),
                            stop=(kd == KD - 1),
                        )
                    nc.vector.tensor_scalar(
                        qT[h * 64 : h * 64 + 64, st * 512 : (st + 1) * 512],
                        ps[0:64, :],
                        bqk_sb[0:64, h : h + 1],
                        None,
                        AOP.add,
                    )
                    nc.vector.tensor_scalar(
                        kT[h * 64 : h * 64 + 64, st * 512 : (st + 1) * 512],
                        ps[64:128, :],
                        bqk_sb[64:128, h : h + 1],
                        None,
                        AOP.add,
                    )
                ps = ps_mm.tile([128, 512], FP, name="ps_v", tag="mm")
                for kd in range(KD):
                    nc.tensor.matmul(
                        ps[:],
                        lhsT=wv_sb[:, kd, :],
                        rhs=xts[kd],
                        start=(kd == 0),
                        stop=(kd == KD - 1),
                    )
                nc.vector.tensor_scalar(
                    vT[:, st * 512 : (st + 1) * 512],
                    ps[:],
                    bv_sb[:, 0:1],
                    None,
                    AOP.add,
                )

            vn_tiles = {}

            def emit_vtrans(b):
                # vT -> v natural [token, hd] tiles with ones columns
                _, _, vT = qkv_tiles[b]
                vn_tiles[b] = []
                for kc in range(S // 128):
                    pst = ps_mm.tile([128, 128], BF, name="ps_t", tag="mm")
                    nc.tensor.transpose(pst[:], vT[:, kc * 128 : (kc + 1) * 128], ident_sb[:])
                    vn = vnp.tile([128, 130], BF, name="vn", tag="vn")
                    nc.vector.tensor_copy(out=vn[:, 0:64], in_=pst[:, 0:64])
                    nc.vector.tensor_copy(out=vn[:, 65:129], in_=pst[:, 64:128])
                    nc.vector.tensor_copy(out=vn[:, 64:65], in_=mask_sb[:, 895:896])
                    nc.vector.tensor_copy(out=vn[:, 129:130], in_=mask_sb[:, 895:896])
                    vn_tiles[b].append(vn)

            def emit_attn_qi(b, qi):
                qT, kT, _ = qkv_tiles[b]
                po = [
                    ps_o.tile([65, 512], FP, name=f"po{h}", tag=f"o{h}")
                    for h in range(2)
                ]
                nki = 4 * (qi + 1)
                for ki in range(nki):
                    j = ki - 4 * qi  # >= 0 on diagonal tiles
                    c0 = 128 * max(j, 0)  # first useful column of this q-tile
                    pss = ps_s.tile([128, 1024], FP, name="ps_sc", tag="sc")
                    for h in range(2):
                        nc.tensor.matmul(
                            pss[:, h * 512 + c0 : (h + 1) * 512],
                            lhsT=kT[h * 64 : h * 64 + 64, ki * 128 : (ki + 1) * 128],
                            rhs=qT[h * 64 : h * 64 + 64, qi * 512 + c0 : (qi + 1) * 512],
                            start=True,
                            stop=True,
                            tile_position=(h * 64, 0),
                        )
                    es = esp.tile([128, 1024], BF, name="es", tag="es")
                    if c0 >= 256:
                        for h in range(2):
                            nc.scalar.activation(
                                es[:, h * 512 + c0 : (h + 1) * 512],
                                pss[:, h * 512 + c0 : (h + 1) * 512],
                                AFT.Exp,
                            )
                    else:
                        nc.scalar.activation(es[:], pss[:], AFT.Exp)
                    if j >= 0:  # diagonal tile: multiplicative causal mask
                        for h in range(2):
                            nc.vector.tensor_tensor(
                                es[:, h * 512 + c0 : (h + 1) * 512],
                                es[:, h * 512 + c0 : (h + 1) * 512],
                                mask_sb[:, 384 : 896 - c0],
                                AOP.mult,
                            )
                    for h in range(2):
                        nc.tensor.matmul(
                            po[h][:, c0:512],
                            lhsT=vn_tiles[b][ki][:, h * 65 : (h + 1) * 65],
                            rhs=es[:, h * 512 + c0 : (h + 1) * 512],
                            start=(ki == 0),
                            stop=(ki == nki - 1),
                        )
                # normalize and scatter into this batch's A2A send buffer
                for h in range(2):
                    oc = small.tile([65, 512], FP, name="oc", tag="oc")
                    nc.vector.tensor_copy(out=oc[:], in_=po[h][:])
                    den = small.tile([1, 512], FP, name="den", tag="den")
                    nc.vector.tensor_copy(out=den[:], in_=oc[64:65, :])
                    bc = small.tile([64, 512], FP, name="bc", tag="bc")
                    nc.gpsimd.partition_broadcast(bc[:], den[0:1, :], channels=64)
                    rc = small.tile([64, 512], FP, name="rc", tag="rc")
                    nc.vector.reciprocal_approx_fast(out=rc[:], in_=bc[:])
                    on = onp.tile([64, 512], BF, name="on", tag="on")
                    nc.vector.tensor_tensor(on[:], oc[0:64, :], rc[:], AOP.mult)
                    t0r = (4 * qi) % 8  # first destination rank of this q-tile
                    nc.sync.dma_start(
                        cc_ins[b][qi // 2][
                            t0r : t0r + 4, h * 64 : (h + 1) * 64, :
                        ].rearrange("r p t -> p r t"),
                        on[:].rearrange("p (r t) -> p r t", r=4),
                    )

            # ---- software-pipelined schedule ----
            # prologue: batch 0 projections
            alloc_qkv(0)
            for st in range(4):
                emit_proj_st(0, st)
            nc.scalar.dma_start(wo_sb[:], wo.rearrange("(ko p) n -> p ko n", p=128))
            emit_vtrans(0)
            for b in range(B):
                if b + 1 < B:
                    alloc_qkv(b + 1)
                for qi in range(4):
                    emit_attn_qi(b, qi)
                    if qi == 1:
                        emit_a2a(b, 0)
                    # interleave next batch's projection work into the
                    # ACT-gated attention stream
                    if b + 1 < B:
                        emit_proj_st(b + 1, qi)
                if b + 1 < B:
                    emit_vtrans(b + 1)
                emit_a2a(b, 1)
                if b >= 1:
                    emit_oproj(b - 1, 0)
                    emit_oproj(b - 1, 1)
            emit_oproj(B - 1, 0)
            emit_oproj(B - 1, 1)

    nc.finalize()
    return nc


_NC_CACHE = None


def _get_nc():
    global _NC_CACHE
    if _NC_CACHE is None:
        _NC_CACHE = build_nc()
    return _NC_CACHE


def make_in_maps(x, Wqkv, bqkv, Wo):
    import ml_dtypes

    bf16 = ml_dtypes.bfloat16
    scale = HD ** -0.5
    xtn = np.ascontiguousarray(x.reshape(NT, D).T).astype(bf16)  # [D, NT]
    mask = (np.arange(896)[None, :] - 384 >= np.arange(128)[:, None]).astype(bf16)
    ident = np.eye(128, dtype=np.float32).astype(bf16)
    wo = np.ascontiguousarray(Wo).astype(bf16)
    in_maps = []
    for c in range(N_CORES):
        h0, h1 = 2 * c, 2 * c + 1
        wqk_c = np.stack(
            [
                np.concatenate(
                    [Wqkv[h][:, 0:64] * scale, Wqkv[h][:, 64:128]], axis=1
                )
                for h in (h0, h1)
            ]
        ).astype(bf16)
        wv_c = np.concatenate(
            [Wqkv[h0][:, 128:192], Wqkv[h1][:, 128:192]], axis=1
        ).astype(bf16)
        bqk_c = np.stack(
            [
                np.concatenate([bqkv[h][0:64] * scale, bqkv[h][64:128]])[:, None]
                for h in (h0, h1)
            ]
        ).astype(np.float32)
        bv_c = np.concatenate([bqkv[h0][128:192], bqkv[h1][128:192]])[:, None].astype(
            np.float32
        )
        in_maps.append(
            {
                "xt": xtn,
                "wqk": np.ascontiguousarray(wqk_c),
                "wv": np.ascontiguousarray(wv_c),
                "bqk": np.ascontiguousarray(bqk_c),
                "bv": np.ascontiguousarray(bv_c),
                "wo": wo,
                "mask": mask,
                "ident": ident,
            }
        )
    return in_maps


def run_cores(in_maps, trace=False, trace_kwargs=None):
    nc = _get_nc()
    kwargs = {}
    if trace:
        kwargs["trace"] = True
        if trace_kwargs:
            kwargs["trace_kwargs"] = trace_kwargs
    return bass_utils.run_bass_kernel_spmd(
        nc, in_maps, core_ids=list(range(N_CORES)), **kwargs
    )


def assemble(results, bo):
    """Reassemble core outputs (interleaved token-tile mapping) into [B,S,D]."""
    full = np.empty((NT, D), np.float32)
    for c in range(N_CORES):
        o = results[c]["out"]
        for b in range(B):
            for pos in range(2):
                t = c + 8 * pos  # token tile within batch
                dst = b * S + t * 128
                full[dst : dst + 128] = o[(2 * b + pos) * 128 : (2 * b + pos + 1) * 128]
    full += bo[None, :]
    return full.reshape(B, S, D)


def kernel(x, Wqkv, bqkv, Wo, bo):
    x = np.asarray(x, dtype=np.float32)
    Wqkv = np.asarray(Wqkv, dtype=np.float32)
    bqkv = np.asarray(bqkv, dtype=np.float32)
    Wo = np.asarray(Wo, dtype=np.float32)
    bo = np.asarray(bo, dtype=np.float32)

    in_maps = make_in_maps(x, Wqkv, bqkv, Wo)
    res = run_cores(in_maps)
    return assemble(res.results, bo)


# revision 20
# speedup vs baseline: 1.4808x; 1.0107x over previous
"""Multi-head causal attention (B=4, S=2048, D=1024, H=16, HD=64) on 8 TRN2 cores.

Strategy:
  - Head-parallel: core i computes heads {2i, 2i+1} for all tokens.
    Host pre-transposes x -> xT [D, B*S], folds the 1/sqrt(HD) scale into Wq,
    converts matmul inputs to bf16, and adds bo at the end.
  - On device per core and per batch: qT/kT/vT projections (weights
    stationary, xT moving), scores computed transposed [k, q] with the two
    heads packed via PE row tiling (K=64 each), exp on ACT, PV matmul with
    stationary [v | 1] so the softmax denominator lands in output row 64,
    normalize via fast reciprocal + gpsimd partition_broadcast.
  - One AllToAll per batch (tokens of that batch, interleaved token-tile ->
    rank mapping) reshards head-outputs feature-major; the output projection
    for those tokens runs overlapped with the next batch's attention.
"""

import sys

sys.path.insert(0, "/opt/trn_rl_repo")

import numpy as np

import concourse.bass as bass
import concourse.mybir as mybir
import concourse.tile as tile
from concourse import bacc, bass_utils

FP = mybir.dt.float32
BF = mybir.dt.bfloat16
AOP = mybir.AluOpType
AFT = mybir.ActivationFunctionType

B, S, D, H = 4, 2048, 1024, 16
HD = 64
N_CORES = 8
NT = B * S  # 8192 tokens
TOK_PER_CORE = NT // N_CORES  # 1024
KD = D // 128  # 8 contraction tiles for the projections


def build_nc():
    nc = bacc.Bacc(None, target_bir_lowering=False, debug=False, num_devices=N_CORES)

    xt = nc.dram_tensor("xt", [16, 128, KD, 512], BF, kind="ExternalInput")
    wqk = nc.dram_tensor("wqk", [128, 2 * KD, 128], BF, kind="ExternalInput")
    wv = nc.dram_tensor("wv", [128, KD, 128], BF, kind="ExternalInput")
    bqk = nc.dram_tensor("bqk", [2, 128, 1], FP, kind="ExternalInput")
    bvb = nc.dram_tensor("bv", [128, 1], FP, kind="ExternalInput")
    wo = nc.dram_tensor("wo", [128, KD, D], BF, kind="ExternalInput")
    maskd = nc.dram_tensor("mask", [128, 896], BF, kind="ExternalInput")
    identd = nc.dram_tensor("ident", [128, 128], BF, kind="ExternalInput")
    out = nc.dram_tensor("out", [TOK_PER_CORE, D], FP, kind="ExternalOutput")

    with tile.TileContext(nc) as tc:
        with (
            tc.tile_pool(name="const", bufs=1) as const,
            tc.tile_pool(name="xtp", bufs=4) as xtp,
            tc.tile_pool(name="qkv", bufs=2) as qkv,
            tc.tile_pool(name="vnp", bufs=18) as vnp,
            tc.tile_pool(name="esp", bufs=4) as esp,
            tc.tile_pool(name="small", bufs=4) as small,
            tc.tile_pool(name="onp", bufs=6) as onp,
            tc.tile_pool(name="actp", bufs=3) as actp,
            tc.tile_pool(name="oop", bufs=3) as oop,
            tc.tile_pool(name="ps_mm", bufs=2, space="PSUM") as ps_mm,
            tc.tile_pool(name="ps_s", bufs=2, space="PSUM") as ps_s,
            tc.tile_pool(name="ps_o", bufs=1, space="PSUM") as ps_o,
            tc.tile_pool(name="dram", bufs=1, space="DRAM") as dram,
        ):
            cc_ins = [
                [
                    dram.tile([N_CORES, 128, 128], BF, name=f"cc_in{b}_{hf}")
                    for hf in range(2)
                ]
                for b in range(B)
            ]
            cc_outs = [
                [
                    dram.tile([N_CORES, 128, 128], BF, name=f"cc_out{b}_{hf}")
                    for hf in range(2)
                ]
                for b in range(B)
            ]

            # ---- resident constants ----
            wqk_sb = const.tile([128, 2 * KD, 128], BF, name="wqk_sb")
            nc.scalar.dma_start(wqk_sb[:], wqk[:])
            wv_sb = const.tile([128, KD, 128], BF, name="wv_sb")
            nc.scalar.dma_start(wv_sb[:], wv[:])
            mask_sb = const.tile([128, 896], BF, name="mask_sb")
            nc.scalar.dma_start(mask_sb[:], maskd[:])
            ident_sb = const.tile([128, 128], BF, name="ident_sb")
            nc.scalar.dma_start(ident_sb[:], identd[:])
            bqk_sb = const.tile([128, 2], FP, name="bqk_sb")
            nc.scalar.dma_start(bqk_sb[:], bqk.rearrange("h p one -> p (h one)"))
            bv_sb = const.tile([128, 1], FP, name="bv_sb")
            nc.scalar.dma_start(bv_sb[:], bvb[:])
            wo_sb = const.tile([128, KD, D], BF, name="wo_sb")

            def emit_oproj(bb, pos):
                # output projection for my token tile (batch bb, half pos)
                at = actp.tile([128, N_CORES, 128], BF, name="at", tag="at")
                nc.sync.dma_start(at[:], cc_outs[bb][pos][:].rearrange("f p t -> p f t"))
                acts = [at[:, ft, :] for ft in range(N_CORES)]
                row0 = (2 * bb + pos) * 128
                for nn in range(2):
                    ps = ps_o.tile([128, 512], FP, name="ps_op", tag=f"o{nn}")
                    for ft in range(N_CORES):
                        nc.tensor.matmul(
                            ps[:],
                            lhsT=acts[ft],
                            rhs=wo_sb[:, ft, nn * 512 : (nn + 1) * 512],
                            start=(ft == 0),
                            stop=(ft == N_CORES - 1),
                        )
                    oo = oop.tile([128, 512], FP, name="oo", tag="oo")
                    nc.vector.tensor_copy(out=oo[:], in_=ps[:])
                    nc.sync.dma_start(
                        out[row0 : row0 + 128, nn * 512 : (nn + 1) * 512], oo[:]
                    )

            def emit_a2a(bb, hf):
                nc.gpsimd.collective_compute(
                    "AllToAll",
                    AOP.bypass,
                    replica_groups=[list(range(N_CORES))],
                    ins=[cc_ins[bb][hf][:].opt()],
                    outs=[cc_outs[bb][hf][:].opt()],
                )

            qkv_tiles = {}

            def alloc_qkv(b):
                qkv_tiles[b] = (
                    qkv.tile([128, S], BF, name="qT", tag="qT"),
                    qkv.tile([128, S], BF, name="kT", tag="kT"),
                    qkv.tile([128, S], BF, name="vT", tag="vT"),
                )

            def emit_proj_st(b, st):
                qT, kT, vT = qkv_tiles[b]
                xt_st = xtp.tile([128, KD, 512], BF, name="xt_st", tag="xt")
                nc.sync.dma_start(xt_st[:], xt[4 * b + st])
                xts = [xt_st[:, kd, :] for kd in range(KD)]
                for h in range(2):
                    ps = ps_mm.tile([128, 512], FP, name="ps_qk", tag="mm")
                    for kd in range(KD):
                        nc.tensor.matmul(
                            ps[:],
                            lhsT=wqk_sb[:, h * KD + kd, :],
                            rhs=xts[kd],
                            start=(kd == 0),
                            stop=(kd == KD - 1),
                        )
                    nc.vector.tensor_scalar(
                        qT[h * 64 : h * 64 + 64, st * 512 : (st + 1) * 512],
                        ps[0:64, :],
                        bqk_sb[0:64, h : h + 1],
                        None,
                        AOP.add,
                    )
                    nc.vector.tensor_scalar(
                        kT[h * 64 : h * 64 + 64, st * 512 : (st + 1) * 512],
                        ps[64:128, :],
                        bqk_sb[64:128, h : h + 1],
                        None,
                        AOP.add,
                    )
                ps = ps_mm.tile([128, 512], FP, name="ps_v", tag="mm")
                for kd in range(KD):
                    nc.tensor.matmul(
                        ps[:],
                        lhsT=wv_sb[:, kd, :],
                        rhs=xts[kd],
                        start=(kd == 0),
                        stop=(kd == KD - 1),
                    )
                nc.vector.tensor_scalar(
                    vT[:, st * 512 : (st + 1) * 512],
                    ps[:],
                    bv_sb[:, 0:1],
                    None,
                    AOP.add,
                )

            vn_tiles = {}

            def emit_vtrans(b):
                # vT -> v natural [token, hd] tiles with ones columns
                _, _, vT = qkv_tiles[b]
                vn_tiles[b] = []
                for kc in range(S // 128):
                    pst = ps_mm.tile([128, 128], BF, name="ps_t", tag="mm")
                    nc.tensor.transpose(pst[:], vT[:, kc * 128 : (kc + 1) * 128], ident_sb[:])
                    vn = vnp.tile([128, 130], BF, name="vn", tag="vn")
                    nc.vector.tensor_copy(out=vn[:, 0:64], in_=pst[:, 0:64])
                    nc.vector.tensor_copy(out=vn[:, 65:129], in_=pst[:, 64:128])
                    nc.vector.tensor_copy(out=vn[:, 64:65], in_=mask_sb[:, 895:896])
                    nc.vector.tensor_copy(out=vn[:, 129:130], in_=mask_sb[:, 895:896])
                    vn_tiles[b].append(vn)

            def emit_attn_qi(b, qi):
                qT, kT, _ = qkv_tiles[b]
                po = [
                    ps_o.tile([65, 512], FP, name=f"po{h}", tag=f"o{h}")
                    for h in range(2)
                ]
                nki = 4 * (qi + 1)
                for ki in range(nki):
                    j = ki - 4 * qi  # >= 0 on diagonal tiles
                    c0 = 128 * max(j, 0)  # first useful column of this q-tile
                    pss = ps_s.tile([128, 1024], FP, name="ps_sc", tag="sc")
                    for h in range(2):
                        nc.tensor.matmul(
                            pss[:, h * 512 + c0 : (h + 1) * 512],
                            lhsT=kT[h * 64 : h * 64 + 64, ki * 128 : (ki + 1) * 128],
                            rhs=qT[h * 64 : h * 64 + 64, qi * 512 + c0 : (qi + 1) * 512],
                            start=True,
                            stop=True,
                            tile_position=(h * 64, 0),
                        )
                    es = esp.tile([128, 1024], BF, name="es", tag="es")
                    if c0 >= 256:
                        for h in range(2):
                            nc.scalar.activation(
                                es[:, h * 512 + c0 : (h + 1) * 512],
                                pss[:, h * 512 + c0 : (h + 1) * 512],
                                AFT.Exp,
                            )
                    else:
                        nc.scalar.activation(es[:], pss[:], AFT.Exp)
                    if j >= 0:  # diagonal tile: multiplicative causal mask
                        for h in range(2):
                            nc.vector.tensor_tensor(
                                es[:, h * 512 + c0 : (h + 1) * 512],
                                es[:, h * 512 + c0 : (h + 1) * 512],
                                mask_sb[:, 384 : 896 - c0],
                                AOP.mult,
                            )
                    for h in range(2):
                        nc.tensor.matmul(
                            po[h][:, c0:512],
                            lhsT=vn_tiles[b][ki][:, h * 65 : (h + 1) * 65],
                            rhs=es[:, h * 512 + c0 : (h + 1) * 512],
                            start=(ki == 0),
                            stop=(ki == nki - 1),
                        )
                # normalize and scatter into this batch's A2A send buffer
                for h in range(2):
                    oc = small.tile([65, 512], FP, name="oc", tag="oc")
                    nc.vector.tensor_copy(out=oc[:], in_=po[h][:])
                    den = small.tile([1, 512], FP, name="den", tag="den")
                    nc.vector.tensor_copy(out=den[:], in_=oc[64:65, :])
                    bc = small.tile([64, 512], FP, name="bc", tag="bc")
                    nc.gpsimd.partition_broadcast(bc[:], den[0:1, :], channels=64)
                    rc = small.tile([64, 512], FP, name="rc", tag="rc")
                    nc.vector.reciprocal_approx_fast(out=rc[:], in_=bc[:])
                    on = onp.tile([64, 512], BF, name="on", tag="on")
                    nc.vector.tensor_tensor(on[:], oc[0:64, :], rc[:], AOP.mult)
                    t0r = (4 * qi) % 8  # first destination rank of this q-tile
                    nc.sync.dma_start(
                        cc_ins[b][qi // 2][
                            t0r : t0r + 4, h * 64 : (h + 1) * 64, :
                        ].rearrange("r p t -> p r t"),
                        on[:].rearrange("p (r t) -> p r t", r=4),
                    )

            # ---- software-pipelined schedule ----
            # prologue: batch 0 projections
            alloc_qkv(0)
            for st in range(4):
                emit_proj_st(0, st)
            nc.scalar.dma_start(wo_sb[:], wo[:])
            emit_vtrans(0)
            for b in range(B):
                if b + 1 < B:
                    alloc_qkv(b + 1)
                for qi in range(4):
                    emit_attn_qi(b, qi)
                    if qi == 1:
                        emit_a2a(b, 0)
                    # interleave next batch's projection work into the
                    # ACT-gated attention stream
                    if b + 1 < B:
                        emit_proj_st(b + 1, qi)
                if b + 1 < B:
                    emit_vtrans(b + 1)
                emit_a2a(b, 1)
                if b >= 1:
                    emit_oproj(b - 1, 0)
                    emit_oproj(b - 1, 1)
            emit_oproj(B - 1, 0)
            emit_oproj(B - 1, 1)

    nc.finalize()
    return nc


_NC_CACHE = None


def _get_nc():
    global _NC_CACHE
    if _NC_CACHE is None:
        _NC_CACHE = build_nc()
    return _NC_CACHE


def make_in_maps(x, Wqkv, bqkv, Wo):
    import ml_dtypes

    bf16 = ml_dtypes.bfloat16
    scale = HD ** -0.5
    xT = x.reshape(NT, D).T.astype(bf16)  # [D, NT]
    xtn = np.ascontiguousarray(
        xT.reshape(KD, 128, 16, 512).transpose(2, 1, 0, 3)
    )  # [slab, p, kd, t]
    mask = (np.arange(896)[None, :] - 384 >= np.arange(128)[:, None]).astype(bf16)
    ident = np.eye(128, dtype=np.float32).astype(bf16)
    wo = np.ascontiguousarray(Wo.astype(bf16).reshape(KD, 128, D).transpose(1, 0, 2))
    in_maps = []
    for c in range(N_CORES):
        h0, h1 = 2 * c, 2 * c + 1
        wqk_c = np.stack(
            [
                np.concatenate(
                    [Wqkv[h][:, 0:64] * scale, Wqkv[h][:, 64:128]], axis=1
                )
                for h in (h0, h1)
            ]
        ).astype(bf16)
        wqk_c = (
            wqk_c.reshape(2, KD, 128, 128).transpose(2, 0, 1, 3).reshape(128, 2 * KD, 128)
        )
        wv_c = np.concatenate(
            [Wqkv[h0][:, 128:192], Wqkv[h1][:, 128:192]], axis=1
        ).astype(bf16)
        wv_c = wv_c.reshape(KD, 128, 128).transpose(1, 0, 2)
        bqk_c = np.stack(
            [
                np.concatenate([bqkv[h][0:64] * scale, bqkv[h][64:128]])[:, None]
                for h in (h0, h1)
            ]
        ).astype(np.float32)
        bv_c = np.concatenate([bqkv[h0][128:192], bqkv[h1][128:192]])[:, None].astype(
            np.float32
        )
        in_maps.append(
            {
                "xt": xtn,
                "wqk": np.ascontiguousarray(wqk_c),
                "wv": np.ascontiguousarray(wv_c),
                "bqk": np.ascontiguousarray(bqk_c),
                "bv": np.ascontiguousarray(bv_c),
                "wo": wo,
                "mask": mask,
                "ident": ident,
            }
        )
    return in_maps


def run_cores(in_maps, trace=False, trace_kwargs=None):
    nc = _get_nc()
    kwargs = {}
    if trace:
        kwargs["trace"] = True
        if trace_kwargs:
            kwargs["trace_kwargs"] = trace_kwargs
    return bass_utils.run_bass_kernel_spmd(
        nc, in_maps, core_ids=list(range(N_CORES)), **kwargs
    )


def assemble(results, bo):
    """Reassemble core outputs (interleaved token-tile mapping) into [B,S,D]."""
    full = np.empty((NT, D), np.float32)
    for c in range(N_CORES):
        o = results[c]["out"]
        for b in range(B):
            for pos in range(2):
                t = c + 8 * pos  # token tile within batch
                dst = b * S + t * 128
                full[dst : dst + 128] = o[(2 * b + pos) * 128 : (2 * b + pos + 1) * 128]
    full += bo[None, :]
    return full.reshape(B, S, D)


def kernel(x, Wqkv, bqkv, Wo, bo):
    x = np.asarray(x, dtype=np.float32)
    Wqkv = np.asarray(Wqkv, dtype=np.float32)
    bqkv = np.asarray(bqkv, dtype=np.float32)
    Wo = np.asarray(Wo, dtype=np.float32)
    bo = np.asarray(bo, dtype=np.float32)

    in_maps = make_in_maps(x, Wqkv, bqkv, Wo)
    res = run_cores(in_maps)
    return assemble(res.results, bo)
